# revision 17
# baseline (speedup 1.0000x reference)
"""Trainium2 Bass kernel for the DimeNet++-style EquivariantInteractionPPBlock.

Strategy (8 NeuronCores, SPMD, no cross-core collectives):
  Triplets are routed on the host to the core owning their id_reduce_ji edge.
  Edges are permuted (host-side bin packing on triplet degree) so that every
  128-edge window receives exactly 1024 triplets -> zero padding and a fixed
  8-block segment-sum schedule per window.  Per-triplet inputs (x0/rbf gathered
  by id_expand_kj, plus sbf) are shipped as fp8-e4m3 streams (validated to be
  within tolerance with bf16 weights); all model math runs on device.  The
  segment-sum is PSUM-accumulated one-hot matmuls; the one-hot comes from an
  iota/is_equal against a host-provided window-local offset.  The summed
  per-window x_kj lives in SBUF between the triplet and edge phases (no DRAM
  round trip).  The per-edge dense stack runs in a second loop, with
  element-wise work split across the Vector and GpSimd engines.

kernel(**inputs) -> np.ndarray [E, 128] float32.
"""
import numpy as np
import ml_dtypes

BF16_NP = ml_dtypes.bfloat16
FP8_NP = ml_dtypes.float8_e4m3
E, T, EMB, INT, NRBF, NSBF = 262144, 2097152, 128, 64, 6, 42
NCORES = 8
EC = E // NCORES
WIN = 128
NWIN = E // WIN          # 2048 global windows
CAP = T // NWIN          # 1024 triplets per window when balanced

_CACHE = {}


# ----------------------------------------------------------------------------
# host-side edge balancing + input routing
# ----------------------------------------------------------------------------

def _balance_edges(deg):
    """Partition edges into NWIN windows of WIN edges with per-window triplet
    degree sums as close to CAP as possible (exactly CAP when achievable).
    Returns wins [NWIN, WIN] edge ids."""
    order = np.argsort(-deg, kind="stable").astype(np.int64)
    mat = order.reshape(WIN, NWIN).copy()
    mat[1::2] = mat[1::2, ::-1]          # serpentine deal
    wins = np.ascontiguousarray(mat.T)   # [NWIN, WIN]
    sums = deg[wins].sum(1)
    # repair: swap single edges between max/min windows until all <= CAP
    for _ in range(20000):
        o = int(np.argmax(sums))
        if sums[o] <= CAP:
            break
        u = int(np.argmin(sums))
        excess = min(sums[o] - CAP, CAP - sums[u])
        do, du = deg[wins[o]], deg[wins[u]]
        diff = do[:, None] - du[None, :]        # [WIN, WIN]
        good = diff[(diff > 0) & (diff <= excess)]
        target = good.max() if good.size else diff[diff > 0].min()
        i, j = np.argwhere(diff == target)[0]
        wins[o, i], wins[u, j] = wins[u, j], wins[o, i]
        d = int(target)
        sums[o] -= d
        sums[u] += d
    return wins, int(sums.max())


def _prep_core_arrays(x0, rbf, sbf, ide, idr):
    deg = np.bincount(idr, minlength=E)
    wins, mx = _balance_edges(deg)
    NBW = max(1, int(np.ceil(mx / 128)))

    winof = np.empty(E, np.int64)
    offof = np.empty(E, np.int64)
    winof[wins] = np.arange(NWIN)[:, None]
    offof[wins] = np.arange(WIN)[None, :]

    wt = winof[idr]                       # triplet -> target window
    sort_idx = np.argsort(wt, kind="stable")
    wt_s = wt[sort_idx]
    counts = np.bincount(wt, minlength=NWIN)
    starts = np.concatenate([[0], np.cumsum(counts)[:-1]])
    within = np.arange(T, dtype=np.int64) - starts[wt_s]
    wincap = NBW * 128
    slots = wt_s * wincap + within        # global padded slot id
    Tglob = NWIN * wincap
    Tpc = Tglob // NCORES

    ide_s = ide[sort_idx]
    if Tglob == T:
        # exact balance: slots == arange(T); direct layout, no scatter
        x0e8 = np.ascontiguousarray(x0[ide_s].T).astype(FP8_NP)
        rbf8 = np.ascontiguousarray(rbf[ide_s].T).astype(FP8_NP)
        sbf8 = np.ascontiguousarray(sbf[sort_idx].T).astype(FP8_NP)
        dl = offof[idr[sort_idx]].astype(np.float32)
    else:
        x0e8 = np.zeros((EMB, Tglob), FP8_NP)
        x0e8[:, slots] = x0[ide_s].T.astype(FP8_NP)
        rbf8 = np.zeros((NRBF, Tglob), FP8_NP)
        rbf8[:, slots] = rbf[ide_s].T.astype(FP8_NP)
        sbf8 = np.zeros((NSBF, Tglob), FP8_NP)
        sbf8[:, slots] = sbf[sort_idx].T.astype(FP8_NP)
        dl = np.full(Tglob, -1.0, np.float32)
        dl[slots] = offof[idr[sort_idx]].astype(np.float32)

    NBpc = Tpc // 128
    dl_t = np.ascontiguousarray(dl.reshape(NWIN * NBW, 128).T)  # [128, NB]

    cores = []
    for c in range(NCORES):
        edges_c = wins[c * (NWIN // NCORES):(c + 1) * (NWIN // NCORES)].ravel()
        sl = slice(c * Tpc, (c + 1) * Tpc)
        cores.append(dict(
            x0e8=np.ascontiguousarray(x0e8[:, sl]),
            rbf8=np.ascontiguousarray(rbf8[:, sl]),
            sbf8=np.ascontiguousarray(sbf8[:, sl]),
            dl=np.ascontiguousarray(dl_t[:, c * NBpc:(c + 1) * NBpc]),
            x0T=np.ascontiguousarray(x0[edges_c].T).astype(BF16_NP),
        ))
    perm = wins.ravel()                   # output row order
    return cores, NBW, perm


def _prep_weights(inputs):
    f32 = np.float32
    Wrbf = (np.asarray(inputs["w_rbf1"], f32) @ np.asarray(inputs["w_rbf2"], f32))
    Wsbf = (np.asarray(inputs["w_sbf1"], f32) @ np.asarray(inputs["w_sbf2"], f32))
    iota = np.tile(np.arange(128, dtype=f32)[None, :], (128, 1)).astype(BF16_NP)
    ident = np.eye(128, dtype=f32)
    bias = np.zeros((128, 8), f32)
    for col, key in enumerate(["b_kj", "b_ji", "bb1", "bb2", "b_fin"]):
        bias[:, col] = np.asarray(inputs[key], f32)
    bias[:, 5] = np.asarray(inputs["ba1"][0], f32)
    bias[:, 6] = np.asarray(inputs["ba2"][0], f32)
    bias[:, 7] = np.asarray(inputs["ba1"][1], f32)
    bias2 = np.asarray(inputs["ba2"][1], f32).reshape(128, 1)
    b16 = lambda a: np.asarray(a, f32).astype(BF16_NP)
    return dict(
        iota=iota,
        wkj=b16(inputs["w_kj"]), wrbf=Wrbf.astype(BF16_NP),
        wdown=b16(inputs["w_down"]), wsbf=Wsbf.astype(BF16_NP),
        ident=ident.astype(BF16_NP),
        wji=b16(inputs["w_ji"]), wup=b16(inputs["w_up"]),
        wb1=b16(inputs["wb1"]), wb2=b16(inputs["wb2"]), wfin=b16(inputs["w_fin"]),
        wa11=b16(inputs["wa1"][0]), wa12=b16(inputs["wa2"][0]),
        wa21=b16(inputs["wa1"][1]), wa22=b16(inputs["wa2"][1]),
        bias=bias, bias2=bias2,
    )


# ----------------------------------------------------------------------------
# bass program
# ----------------------------------------------------------------------------

def build_program(nc, ECa, NBW, WPS=4, repeats=1, unroll=8, act=None, ew=2):
    from concourse import mybir, tile
    from concourse.bass import ts
    F32 = mybir.dt.float32
    BF16 = mybir.dt.bfloat16
    FP8 = mybir.dt.float8e4
    AOP = mybir.AluOpType
    ACTF = mybir.ActivationFunctionType
    if act is None:
        act = ACTF.Silu

    NWC = ECa // 128
    SC = NWC // WPS
    NB = NWC * NBW
    Tpc = NB * 128
    SLOT = WPS * NBW * 128
    EDG = WPS * 128
    BPI = WPS * NBW
    NSUB = SLOT // 512
    SCE = SC // ew
    GRP = 4
    NGRP = BPI // GRP

    def din(name, shape, dt):
        return nc.dram_tensor(name, shape, dt, kind="ExternalInput").ap()

    x0T_d = din("x0T", [128, ECa], BF16)
    x0e8_d = din("x0e8", [128, Tpc], FP8)
    rbf8_d = din("rbf8", [NRBF, Tpc], FP8)
    sbf8_d = din("sbf8", [NSBF, Tpc], FP8)
    dl_d = din("dl", [128, NB], F32)
    iota_d = din("iota", [128, 128], BF16)
    wkj_d = din("wkj", [128, 128], BF16)
    wrbf_d = din("wrbf", [NRBF, 128], BF16)
    wdown_d = din("wdown", [128, INT], BF16)
    wsbf_d = din("wsbf", [NSBF, INT], BF16)
    ident_d = din("ident", [128, 128], BF16)
    wji_d = din("wji", [128, 128], BF16)
    wup_d = din("wup", [INT, 128], BF16)
    wb1_d = din("wb1", [128, 128], BF16)
    wb2_d = din("wb2", [128, 128], BF16)
    wfin_d = din("wfin", [128, 128], BF16)
    wa11_d = din("wa11", [128, 128], BF16)
    wa12_d = din("wa12", [128, 128], BF16)
    wa21_d = din("wa21", [128, 128], BF16)
    wa22_d = din("wa22", [128, 128], BF16)
    bias_d = din("bias", [128, 8], F32)
    bias2_d = din("bias2", [128, 1], F32)
    outT_d = nc.dram_tensor("outT", [128, ECa], BF16, kind="ExternalOutput").ap()

    with tile.TileContext(nc) as tc:
        with (
            tc.tile_pool(name="const", bufs=1) as cp,
            tc.tile_pool(name="tbig", bufs=3) as tb,
            tc.tile_pool(name="small", bufs=4) as sp,
            tc.tile_pool(name="edge", bufs=2) as ep,
            tc.tile_pool(name="pbig", bufs=2, space="PSUM") as pb,
            tc.tile_pool(name="ptr", bufs=1, space="PSUM") as ptp,
            tc.tile_pool(name="pzs", bufs=2, space="PSUM") as pz,
            tc.tile_pool(name="pacc", bufs=1, space="PSUM") as pacc,
            tc.tile_pool(name="prr", bufs=2, space="PSUM") as prp,
        ):
            def cload(d, shape, dt, tag):
                t = cp.tile(shape, dt, tag=tag, name=tag)
                nc.sync.dma_start(out=t[:], in_=d[:, :])
                return t
            iota_c = cload(iota_d, [128, 128], BF16, "iota")
            wkj_c = cload(wkj_d, [128, 128], BF16, "wkj")
            wrbf_c = cload(wrbf_d, [NRBF, 128], BF16, "wrbf")
            wdown_c = cload(wdown_d, [128, INT], BF16, "wdown")
            wsbf_c = cload(wsbf_d, [NSBF, INT], BF16, "wsbf")
            ident_c = cload(ident_d, [128, 128], BF16, "ident")
            wji_c = cload(wji_d, [128, 128], BF16, "wji")
            wup_c = cload(wup_d, [INT, 128], BF16, "wup")
            wb1_c = cload(wb1_d, [128, 128], BF16, "wb1")
            wb2_c = cload(wb2_d, [128, 128], BF16, "wb2")
            wfin_c = cload(wfin_d, [128, 128], BF16, "wfin")
            wa11_c = cload(wa11_d, [128, 128], BF16, "wa11")
            wa12_c = cload(wa12_d, [128, 128], BF16, "wa12")
            wa21_c = cload(wa21_d, [128, 128], BF16, "wa21")
            wa22_c = cload(wa22_d, [128, 128], BF16, "wa22")
            bias_c = cload(bias_d, [128, 8], F32, "bias")
            bias2_c = cload(bias2_d, [128, 1], F32, "bias2")
            bkj = bias_c[:, 0:1]; bji = bias_c[:, 1:2]
            bb1 = bias_c[:, 2:3]; bb2 = bias_c[:, 3:4]; bfin = bias_c[:, 4:5]
            ba11 = bias_c[:, 5:6]; ba12 = bias_c[:, 6:7]; ba21 = bias_c[:, 7:8]
            ba22 = bias2_c[:, 0:1]
            # x_kj segment sums stay in SBUF between the two phases
            xkj_all = cp.tile([128, NWC * INT], BF16, tag="xkj_all",
                              name="xkj_all")

            def body_t(i):
                x0e_t = tb.tile([128, SLOT], FP8, tag="x0e", name="x0e")
                nc.sync.dma_start(out=x0e_t[:], in_=x0e8_d[:, ts(i, SLOT)])
                rbf_t = tb.tile([NRBF, SLOT], FP8, tag="rbfe", name="rbfe")
                nc.sync.dma_start(out=rbf_t[:], in_=rbf8_d[:, ts(i, SLOT)])
                sbf_t = tb.tile([NSBF, SLOT], FP8, tag="sbf", name="sbf")
                nc.sync.dma_start(out=sbf_t[:], in_=sbf8_d[:, ts(i, SLOT)])
                dl_t = sp.tile([128, BPI], F32, tag="dl", name="dl")
                nc.sync.dma_start(out=dl_t[:], in_=dl_d[:, ts(i, BPI)])

                t3 = tb.tile([128, SLOT], BF16, tag="t3", name="t3")
                for j in range(NSUB):
                    sl = slice(j * 512, (j + 1) * 512)
                    pk = pb.tile([128, 512], F32, tag="pbig", name="pk")
                    nc.tensor.matmul(out=pk[:], lhsT=wkj_c[:], rhs=x0e_t[:, sl],
                                     start=True, stop=True)
                    t1 = sp.tile([128, 512], BF16, tag="t1", name="t1")
                    nc.scalar.activation(out=t1[:], in_=pk[:], func=act,
                                         bias=bkj)
                    pr = prp.tile([128, 512], F32, tag="prr", name="pr")
                    nc.tensor.matmul(out=pr[:], lhsT=wrbf_c[:], rhs=rbf_t[:, sl],
                                     start=True, stop=True)
                    nc.vector.tensor_tensor(out=t3[:, sl], in0=t1[:], in1=pr[:],
                                            op=AOP.mult)

                y_t = sp.tile([128, BPI * INT], BF16, tag="y", name="y")
                for g in range(NGRP):
                    pzs = pz.tile([128, 2 * GRP * INT], F32, tag="pzs",
                                  name="pzs")
                    for k in range(GRP):
                        b = g * GRP + k
                        cb = slice(b * 128, (b + 1) * 128)
                        nc.tensor.matmul(out=pzs[:, k * INT:(k + 1) * INT],
                                         lhsT=t3[:, cb], rhs=wdown_c[:],
                                         start=True, stop=True)
                        nc.tensor.matmul(
                            out=pzs[:, (GRP + k) * INT:(GRP + k + 1) * INT],
                            lhsT=sbf_t[:, cb], rhs=wsbf_c[:],
                            start=True, stop=True)
                    zg = sp.tile([128, GRP * INT], BF16, tag="zb", name="zg")
                    nc.scalar.activation(out=zg[:], in_=pzs[:, :GRP * INT],
                                         func=act)
                    nc.vector.tensor_tensor(
                        out=y_t[:, g * GRP * INT:(g + 1) * GRP * INT],
                        in0=zg[:], in1=pzs[:, GRP * INT:], op=AOP.mult)

                acc = pacc.tile([128, WPS * INT], F32, tag="acc", name="acc")
                for w in range(WPS):
                    for k in range(NBW):
                        b = w * NBW + k
                        oh = sp.tile([128, 128], BF16, tag="oh", name="oh")
                        nc.gpsimd.tensor_scalar(out=oh[:], in0=iota_c[:],
                                                scalar1=dl_t[:, b:b + 1],
                                                scalar2=None, op0=AOP.is_equal)
                        nc.tensor.matmul(out=acc[:, w * INT:(w + 1) * INT],
                                         lhsT=oh[:],
                                         rhs=y_t[:, b * INT:(b + 1) * INT],
                                         start=(k == 0), stop=(k == NBW - 1))
                nc.vector.tensor_copy(out=xkj_all[:, ts(i, WPS * INT)],
                                      in_=acc[:])

            def body_e(i):
                def each(fn):
                    for n in range(ew):
                        fn(n)

                x0_t = {}
                for n in range(ew):
                    x0_t[n] = ep.tile([128, EDG], BF16, tag=f"x0_{n}",
                                      name=f"x0_{n}")
                    nc.sync.dma_start(out=x0_t[n][:],
                                      in_=x0T_d[:, ts(i * ew + n, EDG)])

                pjis = {}
                def _ji_mm(n):
                    pjis[n] = pb.tile([128, 512], F32, tag="pbig", name="pji")
                    nc.tensor.matmul(out=pjis[n][:, :EDG], lhsT=wji_c[:],
                                     rhs=x0_t[n][:], start=True, stop=True)
                each(_ji_mm)
                x_ji = {}
                def _ji_act(n):
                    x_ji[n] = ep.tile([128, EDG], BF16, tag=f"xji_{n}",
                                      name=f"xji_{n}")
                    nc.scalar.activation(out=x_ji[n][:], in_=pjis[n][:, :EDG],
                                         func=act, bias=bji)
                each(_ji_act)
                ptrs = {}
                def _tr(n):
                    ptrs[n] = ptp.tile([128, 512], BF16, tag="ptrb", name="ptr")
                    for w in range(WPS):
                        nc.tensor.transpose(
                            out=ptrs[n][:INT, w * 128:(w + 1) * 128],
                            in_=xkj_all[:, ts((i * ew + n) * WPS + w, INT)],
                            identity=ident_c[:])
                each(_tr)
                xkjT = {}
                def _trc(n):
                    xkjT[n] = ep.tile([INT, EDG], BF16, tag=f"xkT_{n}",
                                      name=f"xkT_{n}")
                    nc.vector.tensor_copy(out=xkjT[n][:], in_=ptrs[n][:INT, :EDG])
                each(_trc)
                pus = {}
                def _up_mm(n):
                    pus[n] = pb.tile([128, 512], F32, tag="pbig", name="pup")
                    nc.tensor.matmul(out=pus[n][:, :EDG], lhsT=wup_c[:],
                                     rhs=xkjT[n][:], start=True, stop=True)
                each(_up_mm)
                x2 = {}
                def _up_act(n):
                    xe = ep.tile([128, EDG], BF16, tag=f"xke_{n}", name=f"xke_{n}")
                    nc.scalar.activation(out=xe[:], in_=pus[n][:, :EDG], func=act)
                    x2[n] = ep.tile([128, EDG], BF16, tag=f"x2_{n}",
                                    name=f"x2_{n}")
                    nc.vector.tensor_tensor(out=x2[n][:], in0=x_ji[n][:],
                                            in1=xe[:], op=AOP.add)
                each(_up_act)

                def dense(w_c, srcs, bias_ap, tag):
                    outs = {}
                    ps_ = {}
                    def _mm(n):
                        ps_[n] = pb.tile([128, 512], F32, tag="pbig", name="pd")
                        nc.tensor.matmul(out=ps_[n][:, :EDG], lhsT=w_c[:],
                                         rhs=srcs[n][:], start=True, stop=True)
                    each(_mm)
                    def _act(n):
                        outs[n] = ep.tile([128, EDG], BF16, tag=f"{tag}_{n}",
                                          name=f"{tag}_{n}")
                        nc.scalar.activation(out=outs[n][:], in_=ps_[n][:, :EDG],
                                             func=act, bias=bias_ap)
                    each(_act)
                    return outs

                h1 = dense(wb1_c, x2, bb1, "h")
                h2 = dense(wb2_c, h1, bb2, "i2")
                x2b = {}
                def _add1(n):
                    x2b[n] = ep.tile([128, EDG], BF16, tag=f"x2b_{n}",
                                     name=f"x2b_{n}")
                    nc.vector.tensor_tensor(out=x2b[n][:], in0=x2[n][:],
                                            in1=h2[n][:], op=AOP.add)
                each(_add1)
                x2f = dense(wfin_c, x2b, bfin, "j2")
                x = {}
                def _skip(n):
                    x[n] = ep.tile([128, EDG], BF16, tag=f"x_{n}", name=f"x_{n}")
                    nc.vector.tensor_tensor(out=x[n][:], in0=x0_t[n][:],
                                            in1=x2f[n][:], op=AOP.add)
                each(_skip)
                g1 = dense(wa11_c, x, ba11, "h")
                g2 = dense(wa12_c, g1, ba12, "i2")
                xa = {}
                def _add2(n):
                    xa[n] = ep.tile([128, EDG], BF16, tag=f"xa_{n}",
                                    name=f"xa_{n}")
                    nc.vector.tensor_tensor(out=xa[n][:], in0=x[n][:],
                                            in1=g2[n][:], op=AOP.add)
                each(_add2)
                g3 = dense(wa21_c, xa, ba21, "h")
                g4 = dense(wa22_c, g3, ba22, "i2")
                def _fin(n):
                    xf = ep.tile([128, EDG], BF16, tag=f"xf_{n}", name=f"xf_{n}")
                    nc.gpsimd.tensor_tensor(out=xf[:], in0=xa[n][:],
                                            in1=g4[n][:], op=AOP.add)
                    nc.sync.dma_start(out=outT_d[:, ts(i * ew + n, EDG)],
                                      in_=xf[:])
                each(_fin)

            def run_loop(fn, n_iter):
                if n_iter == 1:
                    fn(0)
                else:
                    tc.For_i_unrolled(0, n_iter, 1, fn,
                                      max_unroll=min(unroll, n_iter))

            def run_all():
                run_loop(body_t, SC)
                # full python unroll: transpose reads of xkj_all need
                # compile-time offsets (no register offsets in ldweights)
                for j in range(SCE):
                    body_e(j)

            if repeats > 1:
                with tc.For_i(0, repeats) as r:
                    run_all()
            else:
                run_all()


# ----------------------------------------------------------------------------
# entry point
# ----------------------------------------------------------------------------

def kernel(**inputs):
    import sys
    if '/opt/trn_rl_repo' not in sys.path:
        sys.path.insert(0, '/opt/trn_rl_repo')
    from concourse import bacc
    from concourse.bass_utils import run_bass_kernel_spmd

    np_inputs = {k: np.asarray(v) for k, v in inputs.items()}
    x0 = np.asarray(np_inputs["x0"], np.float32)
    rbf = np.asarray(np_inputs["rbf"], np.float32)
    sbf = np.asarray(np_inputs["sbf"], np.float32)
    ide = np.asarray(np_inputs["id_expand_kj"], np.int64)
    idr = np.asarray(np_inputs["id_reduce_ji"], np.int64)

    cores, NBW, perm = _prep_core_arrays(x0, rbf, sbf, ide, idr)
    weights = _prep_weights(np_inputs)

    key = ("v3", NBW)
    if key not in _CACHE:
        nc = bacc.Bacc("TRN2", target_bir_lowering=False, debug=False,
                       enable_asserts=True, num_devices=NCORES)
        build_program(nc, ECa=EC, NBW=NBW, WPS=4, repeats=1, unroll=8, ew=2)
        nc.compile()
        _CACHE[key] = nc
    nc = _CACHE[key]

    in_maps = []
    for c in range(NCORES):
        m = dict(cores[c])
        m.update(weights)
        in_maps.append(m)
    res = run_bass_kernel_spmd(nc, in_maps, core_ids=list(range(NCORES)))
    outp = np.concatenate(
        [res.results[c]["outT"].T.astype(np.float32) for c in range(NCORES)],
        axis=0)
    out = np.empty((E, EMB), np.float32)
    out[perm] = outp
    return out


# revision 19
# speedup vs baseline: 2.3965x; 2.3965x over previous
"""Trainium2 Bass kernel for the DimeNet++-style EquivariantInteractionPPBlock.

Strategy (8 NeuronCores, SPMD, no cross-core collectives):
  Triplets are routed on the host to the core owning their id_reduce_ji edge.
  Edges are permuted (host-side bin packing on triplet degree) so that every
  128-edge window receives exactly 1024 triplets -> zero padding and a fixed
  8-block segment-sum schedule per window.  Per-triplet inputs (x0/rbf gathered
  by id_expand_kj, plus sbf) are shipped as fp8-e4m3 streams (validated to be
  within tolerance with bf16 weights); all model math runs on device.  The
  segment-sum is PSUM-accumulated one-hot matmuls; the one-hot comes from an
  iota/is_equal against a host-provided window-local offset.  The summed
  per-window x_kj lives in SBUF between the triplet and edge phases (no DRAM
  round trip).  The per-edge dense stack runs in a second loop, with
  element-wise work split across the Vector and GpSimd engines.

kernel(**inputs) -> np.ndarray [E, 128] float32.
"""
import numpy as np
import ml_dtypes

BF16_NP = ml_dtypes.bfloat16
FP8_NP = ml_dtypes.float8_e4m3
E, T, EMB, INT, NRBF, NSBF = 262144, 2097152, 128, 64, 6, 42
NCORES = 8
EC = E // NCORES
WIN = 128
NWIN = E // WIN          # 2048 global windows
CAP = T // NWIN          # 1024 triplets per window when balanced

_CACHE = {}


# ----------------------------------------------------------------------------
# host-side edge balancing + input routing
# ----------------------------------------------------------------------------

def _balance_edges(deg):
    """Partition edges into NWIN windows of WIN edges with per-window triplet
    degree sums as close to CAP as possible (exactly CAP when achievable).
    Returns wins [NWIN, WIN] edge ids."""
    order = np.argsort(-deg, kind="stable").astype(np.int64)
    mat = order.reshape(WIN, NWIN).copy()
    mat[1::2] = mat[1::2, ::-1]          # serpentine deal
    wins = np.ascontiguousarray(mat.T)   # [NWIN, WIN]
    sums = deg[wins].sum(1)
    # repair: swap single edges between max/min windows until all <= CAP
    for _ in range(20000):
        o = int(np.argmax(sums))
        if sums[o] <= CAP:
            break
        u = int(np.argmin(sums))
        excess = min(sums[o] - CAP, CAP - sums[u])
        do, du = deg[wins[o]], deg[wins[u]]
        diff = do[:, None] - du[None, :]        # [WIN, WIN]
        good = diff[(diff > 0) & (diff <= excess)]
        target = good.max() if good.size else diff[diff > 0].min()
        i, j = np.argwhere(diff == target)[0]
        wins[o, i], wins[u, j] = wins[u, j], wins[o, i]
        d = int(target)
        sums[o] -= d
        sums[u] += d
    return wins, int(sums.max())


def _prep_core_arrays(x0, rbf, sbf, ide, idr):
    deg = np.bincount(idr, minlength=E)
    wins, mx = _balance_edges(deg)
    NBW = max(1, int(np.ceil(mx / 128)))

    winof = np.empty(E, np.int64)
    offof = np.empty(E, np.int64)
    winof[wins] = np.arange(NWIN)[:, None]
    offof[wins] = np.arange(WIN)[None, :]

    wt = winof[idr]                       # triplet -> target window
    sort_idx = np.argsort(wt, kind="stable")
    wt_s = wt[sort_idx]
    counts = np.bincount(wt, minlength=NWIN)
    starts = np.concatenate([[0], np.cumsum(counts)[:-1]])
    within = np.arange(T, dtype=np.int64) - starts[wt_s]
    wincap = NBW * 128
    slots = wt_s * wincap + within        # global padded slot id
    Tglob = NWIN * wincap
    Tpc = Tglob // NCORES

    ide_s = ide[sort_idx]
    if Tglob == T:
        # exact balance: slots == arange(T); direct layout, no scatter
        x0e8 = np.ascontiguousarray(x0[ide_s].T).astype(FP8_NP)
        rbf8 = np.ascontiguousarray(rbf[ide_s].T).astype(FP8_NP)
        sbf8 = np.ascontiguousarray(sbf[sort_idx].T).astype(FP8_NP)
        dl = offof[idr[sort_idx]].astype(np.float32)
    else:
        x0e8 = np.zeros((EMB, Tglob), FP8_NP)
        x0e8[:, slots] = x0[ide_s].T.astype(FP8_NP)
        rbf8 = np.zeros((NRBF, Tglob), FP8_NP)
        rbf8[:, slots] = rbf[ide_s].T.astype(FP8_NP)
        sbf8 = np.zeros((NSBF, Tglob), FP8_NP)
        sbf8[:, slots] = sbf[sort_idx].T.astype(FP8_NP)
        dl = np.full(Tglob, -1.0, np.float32)
        dl[slots] = offof[idr[sort_idx]].astype(np.float32)

    NBpc = Tpc // 128
    dl_t = np.ascontiguousarray(dl.reshape(NWIN * NBW, 128).T)  # [128, NB]

    cores = []
    for c in range(NCORES):
        edges_c = wins[c * (NWIN // NCORES):(c + 1) * (NWIN // NCORES)].ravel()
        sl = slice(c * Tpc, (c + 1) * Tpc)
        cores.append(dict(
            x0e8=np.ascontiguousarray(x0e8[:, sl]),
            rbf8=np.ascontiguousarray(rbf8[:, sl]),
            sbf8=np.ascontiguousarray(sbf8[:, sl]),
            dl=np.ascontiguousarray(dl_t[:, c * NBpc:(c + 1) * NBpc]),
            x0T=np.ascontiguousarray(x0[edges_c].T).astype(BF16_NP),
        ))
    perm = wins.ravel()                   # output row order
    return cores, NBW, perm


def _prep_weights(inputs):
    f32 = np.float32
    Wrbf = (np.asarray(inputs["w_rbf1"], f32) @ np.asarray(inputs["w_rbf2"], f32))
    Wsbf = (np.asarray(inputs["w_sbf1"], f32) @ np.asarray(inputs["w_sbf2"], f32))
    iota = np.tile(np.arange(128, dtype=f32)[None, :], (128, 1)).astype(BF16_NP)
    ident = np.eye(128, dtype=f32)
    bias = np.zeros((128, 8), f32)
    for col, key in enumerate(["b_kj", "b_ji", "bb1", "bb2", "b_fin"]):
        bias[:, col] = np.asarray(inputs[key], f32)
    bias[:, 5] = np.asarray(inputs["ba1"][0], f32)
    bias[:, 6] = np.asarray(inputs["ba2"][0], f32)
    bias[:, 7] = np.asarray(inputs["ba1"][1], f32)
    bias2 = np.asarray(inputs["ba2"][1], f32).reshape(128, 1)
    b16 = lambda a: np.asarray(a, f32).astype(BF16_NP)
    return dict(
        iota=iota,
        wkj=b16(inputs["w_kj"]), wrbf=Wrbf.astype(BF16_NP),
        wdown=b16(inputs["w_down"]), wsbf=Wsbf.astype(BF16_NP),
        ident=ident.astype(BF16_NP),
        wji=b16(inputs["w_ji"]), wup=b16(inputs["w_up"]),
        wb1=b16(inputs["wb1"]), wb2=b16(inputs["wb2"]), wfin=b16(inputs["w_fin"]),
        wa11=b16(inputs["wa1"][0]), wa12=b16(inputs["wa2"][0]),
        wa21=b16(inputs["wa1"][1]), wa22=b16(inputs["wa2"][1]),
        bias=bias, bias2=bias2,
    )


# ----------------------------------------------------------------------------
# bass program
# ----------------------------------------------------------------------------

def build_program(nc, ECa, NBW, WPS=4, repeats=1, unroll=8, act=None, ew=2):
    from concourse import mybir, tile
    from concourse.bass import ts
    F32 = mybir.dt.float32
    BF16 = mybir.dt.bfloat16
    FP8 = mybir.dt.float8e4
    AOP = mybir.AluOpType
    ACTF = mybir.ActivationFunctionType
    if act is None:
        act = ACTF.Silu

    NWC = ECa // 128
    SC = NWC // WPS
    NB = NWC * NBW
    Tpc = NB * 128
    SLOT = WPS * NBW * 128
    EDG = WPS * 128
    BPI = WPS * NBW
    NSUB = SLOT // 512
    SCE = SC // ew
    GRP = 4
    NGRP = BPI // GRP

    def din(name, shape, dt):
        return nc.dram_tensor(name, shape, dt, kind="ExternalInput").ap()

    x0T_d = din("x0T", [128, ECa], BF16)
    x0e8_d = din("x0e8", [128, Tpc], FP8)
    rbf8_d = din("rbf8", [NRBF, Tpc], FP8)
    sbf8_d = din("sbf8", [NSBF, Tpc], FP8)
    dl_d = din("dl", [128, NB], F32)
    iota_d = din("iota", [128, 128], BF16)
    wkj_d = din("wkj", [128, 128], BF16)
    wrbf_d = din("wrbf", [NRBF, 128], BF16)
    wdown_d = din("wdown", [128, INT], BF16)
    wsbf_d = din("wsbf", [NSBF, INT], BF16)
    ident_d = din("ident", [128, 128], BF16)
    wji_d = din("wji", [128, 128], BF16)
    wup_d = din("wup", [INT, 128], BF16)
    wb1_d = din("wb1", [128, 128], BF16)
    wb2_d = din("wb2", [128, 128], BF16)
    wfin_d = din("wfin", [128, 128], BF16)
    wa11_d = din("wa11", [128, 128], BF16)
    wa12_d = din("wa12", [128, 128], BF16)
    wa21_d = din("wa21", [128, 128], BF16)
    wa22_d = din("wa22", [128, 128], BF16)
    bias_d = din("bias", [128, 8], F32)
    bias2_d = din("bias2", [128, 1], F32)
    outT_d = nc.dram_tensor("outT", [128, ECa], BF16, kind="ExternalOutput").ap()

    with tile.TileContext(nc) as tc:
        with (
            tc.tile_pool(name="const", bufs=1) as cp,
            tc.tile_pool(name="tbig", bufs=3) as tb,
            tc.tile_pool(name="small", bufs=4) as sp,
            tc.tile_pool(name="edge", bufs=2) as ep,
            tc.tile_pool(name="pbig", bufs=2, space="PSUM") as pb,
            tc.tile_pool(name="ptr", bufs=1, space="PSUM") as ptp,
            tc.tile_pool(name="pzs", bufs=2, space="PSUM") as pz,
            tc.tile_pool(name="pacc", bufs=1, space="PSUM") as pacc,
            tc.tile_pool(name="prr", bufs=2, space="PSUM") as prp,
        ):
            def cload(d, shape, dt, tag):
                t = cp.tile(shape, dt, tag=tag, name=tag)
                nc.sync.dma_start(out=t[:], in_=d[:, :])
                return t
            iota_c = cload(iota_d, [128, 128], BF16, "iota")
            wkj_c = cload(wkj_d, [128, 128], BF16, "wkj")
            wrbf_c = cload(wrbf_d, [NRBF, 128], BF16, "wrbf")
            wdown_c = cload(wdown_d, [128, INT], BF16, "wdown")
            wsbf_c = cload(wsbf_d, [NSBF, INT], BF16, "wsbf")
            ident_c = cload(ident_d, [128, 128], BF16, "ident")
            wji_c = cload(wji_d, [128, 128], BF16, "wji")
            wup_c = cload(wup_d, [INT, 128], BF16, "wup")
            wb1_c = cload(wb1_d, [128, 128], BF16, "wb1")
            wb2_c = cload(wb2_d, [128, 128], BF16, "wb2")
            wfin_c = cload(wfin_d, [128, 128], BF16, "wfin")
            wa11_c = cload(wa11_d, [128, 128], BF16, "wa11")
            wa12_c = cload(wa12_d, [128, 128], BF16, "wa12")
            wa21_c = cload(wa21_d, [128, 128], BF16, "wa21")
            wa22_c = cload(wa22_d, [128, 128], BF16, "wa22")
            bias_c = cload(bias_d, [128, 8], F32, "bias")
            bias2_c = cload(bias2_d, [128, 1], F32, "bias2")
            bkj = bias_c[:, 0:1]; bji = bias_c[:, 1:2]
            bb1 = bias_c[:, 2:3]; bb2 = bias_c[:, 3:4]; bfin = bias_c[:, 4:5]
            ba11 = bias_c[:, 5:6]; ba12 = bias_c[:, 6:7]; ba21 = bias_c[:, 7:8]
            ba22 = bias2_c[:, 0:1]
            # x_kj segment sums stay in SBUF between the two phases
            xkj_all = cp.tile([128, NWC * INT], BF16, tag="xkj_all",
                              name="xkj_all")

            def body_t(i):
                x0e_t = tb.tile([128, SLOT], FP8, tag="x0e", name="x0e")
                nc.sync.dma_start(out=x0e_t[:], in_=x0e8_d[:, ts(i, SLOT)])
                rbf_t = tb.tile([NRBF, SLOT], FP8, tag="rbfe", name="rbfe")
                nc.sync.dma_start(out=rbf_t[:], in_=rbf8_d[:, ts(i, SLOT)])
                sbf_t = tb.tile([NSBF, SLOT], FP8, tag="sbf", name="sbf")
                nc.sync.dma_start(out=sbf_t[:], in_=sbf8_d[:, ts(i, SLOT)])
                dl_t = sp.tile([128, BPI], F32, tag="dl", name="dl")
                nc.sync.dma_start(out=dl_t[:], in_=dl_d[:, ts(i, BPI)])

                t3 = tb.tile([128, SLOT], BF16, tag="t3", name="t3")
                for j in range(NSUB):
                    sl = slice(j * 512, (j + 1) * 512)
                    pk = pb.tile([128, 512], F32, tag="pbig", name="pk")
                    nc.tensor.matmul(out=pk[:], lhsT=wkj_c[:], rhs=x0e_t[:, sl],
                                     start=True, stop=True)
                    t1 = sp.tile([128, 512], BF16, tag="t1", name="t1")
                    nc.scalar.activation(out=t1[:], in_=pk[:], func=act,
                                         bias=bkj)
                    pr = prp.tile([128, 512], F32, tag="prr", name="pr")
                    nc.tensor.matmul(out=pr[:], lhsT=wrbf_c[:], rhs=rbf_t[:, sl],
                                     start=True, stop=True)
                    nc.vector.tensor_tensor(out=t3[:, sl], in0=t1[:], in1=pr[:],
                                            op=AOP.mult)

                y_t = sp.tile([128, BPI * INT], BF16, tag="y", name="y")
                for g in range(NGRP):
                    pzs = pz.tile([128, 2 * GRP * INT], F32, tag="pzs",
                                  name="pzs")
                    for k in range(GRP):
                        b = g * GRP + k
                        cb = slice(b * 128, (b + 1) * 128)
                        nc.tensor.matmul(out=pzs[:, k * INT:(k + 1) * INT],
                                         lhsT=t3[:, cb], rhs=wdown_c[:],
                                         start=True, stop=True)
                        nc.tensor.matmul(
                            out=pzs[:, (GRP + k) * INT:(GRP + k + 1) * INT],
                            lhsT=sbf_t[:, cb], rhs=wsbf_c[:],
                            start=True, stop=True)
                    zg = sp.tile([128, GRP * INT], BF16, tag="zb", name="zg")
                    nc.scalar.activation(out=zg[:], in_=pzs[:, :GRP * INT],
                                         func=act)
                    nc.vector.tensor_tensor(
                        out=y_t[:, g * GRP * INT:(g + 1) * GRP * INT],
                        in0=zg[:], in1=pzs[:, GRP * INT:], op=AOP.mult)

                acc = pacc.tile([128, WPS * INT], F32, tag="acc", name="acc")
                for w in range(WPS):
                    for k in range(NBW):
                        b = w * NBW + k
                        oh = sp.tile([128, 128], BF16, tag="oh", name="oh")
                        nc.vector.tensor_scalar(out=oh[:], in0=iota_c[:],
                                                scalar1=dl_t[:, b:b + 1],
                                                scalar2=None, op0=AOP.is_equal)
                        nc.tensor.matmul(out=acc[:, w * INT:(w + 1) * INT],
                                         lhsT=oh[:],
                                         rhs=y_t[:, b * INT:(b + 1) * INT],
                                         start=(k == 0), stop=(k == NBW - 1))
                nc.vector.tensor_copy(out=xkj_all[:, ts(i, WPS * INT)],
                                      in_=acc[:])

            def body_e(i):
                def each(fn):
                    for n in range(ew):
                        fn(n)

                x0_t = {}
                for n in range(ew):
                    x0_t[n] = ep.tile([128, EDG], BF16, tag=f"x0_{n}",
                                      name=f"x0_{n}")
                    nc.sync.dma_start(out=x0_t[n][:],
                                      in_=x0T_d[:, ts(i * ew + n, EDG)])

                pjis = {}
                def _ji_mm(n):
                    pjis[n] = pb.tile([128, 512], F32, tag="pbig", name="pji")
                    nc.tensor.matmul(out=pjis[n][:, :EDG], lhsT=wji_c[:],
                                     rhs=x0_t[n][:], start=True, stop=True)
                each(_ji_mm)
                x_ji = {}
                def _ji_act(n):
                    x_ji[n] = ep.tile([128, EDG], BF16, tag=f"xji_{n}",
                                      name=f"xji_{n}")
                    nc.scalar.activation(out=x_ji[n][:], in_=pjis[n][:, :EDG],
                                         func=act, bias=bji)
                each(_ji_act)
                ptrs = {}
                def _tr(n):
                    ptrs[n] = ptp.tile([128, 512], BF16, tag="ptrb", name="ptr")
                    for w in range(WPS):
                        nc.tensor.transpose(
                            out=ptrs[n][:INT, w * 128:(w + 1) * 128],
                            in_=xkj_all[:, ts((i * ew + n) * WPS + w, INT)],
                            identity=ident_c[:])
                each(_tr)
                xkjT = {}
                def _trc(n):
                    xkjT[n] = ep.tile([INT, EDG], BF16, tag=f"xkT_{n}",
                                      name=f"xkT_{n}")
                    nc.vector.tensor_copy(out=xkjT[n][:], in_=ptrs[n][:INT, :EDG])
                each(_trc)
                pus = {}
                def _up_mm(n):
                    pus[n] = pb.tile([128, 512], F32, tag="pbig", name="pup")
                    nc.tensor.matmul(out=pus[n][:, :EDG], lhsT=wup_c[:],
                                     rhs=xkjT[n][:], start=True, stop=True)
                each(_up_mm)
                x2 = {}
                def _up_act(n):
                    xe = ep.tile([128, EDG], BF16, tag=f"xke_{n}", name=f"xke_{n}")
                    nc.scalar.activation(out=xe[:], in_=pus[n][:, :EDG], func=act)
                    x2[n] = ep.tile([128, EDG], BF16, tag=f"x2_{n}",
                                    name=f"x2_{n}")
                    nc.vector.tensor_tensor(out=x2[n][:], in0=x_ji[n][:],
                                            in1=xe[:], op=AOP.add)
                each(_up_act)

                def dense(w_c, srcs, bias_ap, tag):
                    outs = {}
                    ps_ = {}
                    def _mm(n):
                        ps_[n] = pb.tile([128, 512], F32, tag="pbig", name="pd")
                        nc.tensor.matmul(out=ps_[n][:, :EDG], lhsT=w_c[:],
                                         rhs=srcs[n][:], start=True, stop=True)
                    each(_mm)
                    def _act(n):
                        outs[n] = ep.tile([128, EDG], BF16, tag=f"{tag}_{n}",
                                          name=f"{tag}_{n}")
                        nc.scalar.activation(out=outs[n][:], in_=ps_[n][:, :EDG],
                                             func=act, bias=bias_ap)
                    each(_act)
                    return outs

                h1 = dense(wb1_c, x2, bb1, "h")
                h2 = dense(wb2_c, h1, bb2, "i2")
                x2b = {}
                def _add1(n):
                    x2b[n] = ep.tile([128, EDG], BF16, tag=f"x2b_{n}",
                                     name=f"x2b_{n}")
                    nc.vector.tensor_tensor(out=x2b[n][:], in0=x2[n][:],
                                            in1=h2[n][:], op=AOP.add)
                each(_add1)
                x2f = dense(wfin_c, x2b, bfin, "j2")
                x = {}
                def _skip(n):
                    x[n] = ep.tile([128, EDG], BF16, tag=f"x_{n}", name=f"x_{n}")
                    nc.vector.tensor_tensor(out=x[n][:], in0=x0_t[n][:],
                                            in1=x2f[n][:], op=AOP.add)
                each(_skip)
                g1 = dense(wa11_c, x, ba11, "h")
                g2 = dense(wa12_c, g1, ba12, "i2")
                xa = {}
                def _add2(n):
                    xa[n] = ep.tile([128, EDG], BF16, tag=f"xa_{n}",
                                    name=f"xa_{n}")
                    nc.vector.tensor_tensor(out=xa[n][:], in0=x[n][:],
                                            in1=g2[n][:], op=AOP.add)
                each(_add2)
                g3 = dense(wa21_c, xa, ba21, "h")
                g4 = dense(wa22_c, g3, ba22, "i2")
                def _fin(n):
                    xf = ep.tile([128, EDG], BF16, tag=f"xf_{n}", name=f"xf_{n}")
                    nc.vector.tensor_tensor(out=xf[:], in0=xa[n][:],
                                            in1=g4[n][:], op=AOP.add)
                    nc.sync.dma_start(out=outT_d[:, ts(i * ew + n, EDG)],
                                      in_=xf[:])
                each(_fin)

            def run_loop(fn, n_iter):
                if n_iter == 1:
                    fn(0)
                else:
                    tc.For_i_unrolled(0, n_iter, 1, fn,
                                      max_unroll=min(unroll, n_iter))

            def run_all():
                run_loop(body_t, SC)
                # full python unroll: transpose reads of xkj_all need
                # compile-time offsets (no register offsets in ldweights)
                for j in range(SCE):
                    body_e(j)

            if repeats > 1:
                with tc.For_i(0, repeats) as r:
                    run_all()
            else:
                run_all()


# ----------------------------------------------------------------------------
# entry point
# ----------------------------------------------------------------------------

def kernel(**inputs):
    import sys
    if '/opt/trn_rl_repo' not in sys.path:
        sys.path.insert(0, '/opt/trn_rl_repo')
    from concourse import bacc
    from concourse.bass_utils import run_bass_kernel_spmd

    np_inputs = {k: np.asarray(v) for k, v in inputs.items()}
    x0 = np.asarray(np_inputs["x0"], np.float32)
    rbf = np.asarray(np_inputs["rbf"], np.float32)
    sbf = np.asarray(np_inputs["sbf"], np.float32)
    ide = np.asarray(np_inputs["id_expand_kj"], np.int64)
    idr = np.asarray(np_inputs["id_reduce_ji"], np.int64)

    cores, NBW, perm = _prep_core_arrays(x0, rbf, sbf, ide, idr)
    weights = _prep_weights(np_inputs)

    key = ("v3", NBW)
    if key not in _CACHE:
        nc = bacc.Bacc("TRN2", target_bir_lowering=False, debug=False,
                       enable_asserts=True, num_devices=NCORES)
        build_program(nc, ECa=EC, NBW=NBW, WPS=4, repeats=1, unroll=8, ew=2)
        nc.compile()
        _CACHE[key] = nc
    nc = _CACHE[key]

    in_maps = []
    for c in range(NCORES):
        m = dict(cores[c])
        m.update(weights)
        in_maps.append(m)
    res = run_bass_kernel_spmd(nc, in_maps, core_ids=list(range(NCORES)))
    outp = np.concatenate(
        [res.results[c]["outT"].T.astype(np.float32) for c in range(NCORES)],
        axis=0)
    out = np.empty((E, EMB), np.float32)
    out[perm] = outp
    return out


# revision 20
# speedup vs baseline: 2.8221x; 1.1776x over previous
"""Trainium2 Bass kernel for the DimeNet++-style EquivariantInteractionPPBlock.

Strategy (8 NeuronCores, SPMD, no cross-core collectives):
  Triplets are routed on the host to the core owning their id_reduce_ji edge.
  Edges are permuted (host-side bin packing on triplet degree) so that every
  128-edge window receives exactly 1024 triplets -> zero padding and a fixed
  8-block segment-sum schedule per window.  Per-triplet inputs (x0/rbf gathered
  by id_expand_kj, plus sbf) are shipped as fp8-e4m3 streams (validated to be
  within tolerance with bf16 weights); all model math runs on device.  The
  segment-sum is PSUM-accumulated one-hot matmuls; the one-hot comes from an
  iota/is_equal against a host-provided window-local offset.  The summed
  per-window x_kj lives in SBUF between the triplet and edge phases (no DRAM
  round trip).  The per-edge dense stack runs in a second loop, with
  element-wise work split across the Vector and GpSimd engines.

kernel(**inputs) -> np.ndarray [E, 128] float32.
"""
import numpy as np
import ml_dtypes

BF16_NP = ml_dtypes.bfloat16
FP8_NP = ml_dtypes.float8_e4m3
E, T, EMB, INT, NRBF, NSBF = 262144, 2097152, 128, 64, 6, 42
NCORES = 8
EC = E // NCORES
WIN = 128
NWIN = E // WIN          # 2048 global windows
CAP = T // NWIN          # 1024 triplets per window when balanced

_CACHE = {}


# ----------------------------------------------------------------------------
# host-side edge balancing + input routing
# ----------------------------------------------------------------------------

def _balance_edges(deg):
    """Partition edges into NWIN windows of WIN edges with per-window triplet
    degree sums as close to CAP as possible (exactly CAP when achievable).
    Returns wins [NWIN, WIN] edge ids."""
    order = np.argsort(-deg, kind="stable").astype(np.int64)
    mat = order.reshape(WIN, NWIN).copy()
    mat[1::2] = mat[1::2, ::-1]          # serpentine deal
    wins = np.ascontiguousarray(mat.T)   # [NWIN, WIN]
    sums = deg[wins].sum(1)
    # repair: swap single edges between max/min windows until all <= CAP
    for _ in range(20000):
        o = int(np.argmax(sums))
        if sums[o] <= CAP:
            break
        u = int(np.argmin(sums))
        excess = min(sums[o] - CAP, CAP - sums[u])
        do, du = deg[wins[o]], deg[wins[u]]
        diff = do[:, None] - du[None, :]        # [WIN, WIN]
        good = diff[(diff > 0) & (diff <= excess)]
        target = good.max() if good.size else diff[diff > 0].min()
        i, j = np.argwhere(diff == target)[0]
        wins[o, i], wins[u, j] = wins[u, j], wins[o, i]
        d = int(target)
        sums[o] -= d
        sums[u] += d
    return wins, int(sums.max())


def _prep_core_arrays(x0, rbf, sbf, ide, idr):
    deg = np.bincount(idr, minlength=E)
    wins, mx = _balance_edges(deg)
    NBW = max(1, int(np.ceil(mx / 128)))

    winof = np.empty(E, np.int64)
    offof = np.empty(E, np.int64)
    winof[wins] = np.arange(NWIN)[:, None]
    offof[wins] = np.arange(WIN)[None, :]

    wt = winof[idr]                       # triplet -> target window
    sort_idx = np.argsort(wt, kind="stable")
    wt_s = wt[sort_idx]
    counts = np.bincount(wt, minlength=NWIN)
    starts = np.concatenate([[0], np.cumsum(counts)[:-1]])
    within = np.arange(T, dtype=np.int64) - starts[wt_s]
    wincap = NBW * 128
    slots = wt_s * wincap + within        # global padded slot id
    Tglob = NWIN * wincap
    Tpc = Tglob // NCORES

    ide_s = ide[sort_idx]
    if Tglob == T:
        # exact balance: slots == arange(T); direct layout, no scatter
        x0e8 = np.ascontiguousarray(x0[ide_s].T).astype(FP8_NP)
        rbf8 = np.ascontiguousarray(rbf[ide_s].T).astype(FP8_NP)
        sbf8 = np.ascontiguousarray(sbf[sort_idx].T).astype(FP8_NP)
        dl = offof[idr[sort_idx]].astype(np.float32)
    else:
        x0e8 = np.zeros((EMB, Tglob), FP8_NP)
        x0e8[:, slots] = x0[ide_s].T.astype(FP8_NP)
        rbf8 = np.zeros((NRBF, Tglob), FP8_NP)
        rbf8[:, slots] = rbf[ide_s].T.astype(FP8_NP)
        sbf8 = np.zeros((NSBF, Tglob), FP8_NP)
        sbf8[:, slots] = sbf[sort_idx].T.astype(FP8_NP)
        dl = np.full(Tglob, -1.0, np.float32)
        dl[slots] = offof[idr[sort_idx]].astype(np.float32)

    NBpc = Tpc // 128
    dl_t = np.ascontiguousarray(dl.reshape(NWIN * NBW, 128).T)  # [128, NB]

    cores = []
    for c in range(NCORES):
        edges_c = wins[c * (NWIN // NCORES):(c + 1) * (NWIN // NCORES)].ravel()
        sl = slice(c * Tpc, (c + 1) * Tpc)
        cores.append(dict(
            x0e8=np.ascontiguousarray(x0e8[:, sl]),
            rbf8=np.ascontiguousarray(rbf8[:, sl]),
            sbf8=np.ascontiguousarray(sbf8[:, sl]),
            dl=np.ascontiguousarray(dl_t[:, c * NBpc:(c + 1) * NBpc]),
            x0T=np.ascontiguousarray(x0[edges_c].T).astype(BF16_NP),
        ))
    perm = wins.ravel()                   # output row order
    return cores, NBW, perm


def _prep_weights(inputs):
    f32 = np.float32
    Wrbf = (np.asarray(inputs["w_rbf1"], f32) @ np.asarray(inputs["w_rbf2"], f32))
    Wsbf = (np.asarray(inputs["w_sbf1"], f32) @ np.asarray(inputs["w_sbf2"], f32))
    iota = np.tile(np.arange(128, dtype=f32)[None, :], (128, 1)).astype(BF16_NP)
    ident = np.eye(128, dtype=f32)
    bias = np.zeros((128, 8), f32)
    for col, key in enumerate(["b_kj", "b_ji", "bb1", "bb2", "b_fin"]):
        bias[:, col] = np.asarray(inputs[key], f32)
    bias[:, 5] = np.asarray(inputs["ba1"][0], f32)
    bias[:, 6] = np.asarray(inputs["ba2"][0], f32)
    bias[:, 7] = np.asarray(inputs["ba1"][1], f32)
    bias2 = np.asarray(inputs["ba2"][1], f32).reshape(128, 1)
    b16 = lambda a: np.asarray(a, f32).astype(BF16_NP)
    return dict(
        iota=iota,
        wkj=b16(inputs["w_kj"]), wrbf=Wrbf.astype(BF16_NP),
        wdown=b16(inputs["w_down"]), wsbf=Wsbf.astype(BF16_NP),
        ident=ident.astype(BF16_NP),
        wji=b16(inputs["w_ji"]), wup=b16(inputs["w_up"]),
        wb1=b16(inputs["wb1"]), wb2=b16(inputs["wb2"]), wfin=b16(inputs["w_fin"]),
        wa11=b16(inputs["wa1"][0]), wa12=b16(inputs["wa2"][0]),
        wa21=b16(inputs["wa1"][1]), wa22=b16(inputs["wa2"][1]),
        bias=bias, bias2=bias2,
    )


# ----------------------------------------------------------------------------
# bass program
# ----------------------------------------------------------------------------

def build_program(nc, ECa, NBW, WPS=4, repeats=1, unroll=8, act=None, ew=2):
    from concourse import mybir, tile
    from concourse.bass import ts
    F32 = mybir.dt.float32
    BF16 = mybir.dt.bfloat16
    FP8 = mybir.dt.float8e4
    AOP = mybir.AluOpType
    ACTF = mybir.ActivationFunctionType
    if act is None:
        act = ACTF.Silu

    NWC = ECa // 128
    SC = NWC // WPS
    NB = NWC * NBW
    Tpc = NB * 128
    SLOT = WPS * NBW * 128
    EDG = WPS * 128
    BPI = WPS * NBW
    NSUB = SLOT // 512
    SCE = SC // ew
    GRP = 8
    NGRP = BPI // GRP

    def din(name, shape, dt):
        return nc.dram_tensor(name, shape, dt, kind="ExternalInput").ap()

    x0T_d = din("x0T", [128, ECa], BF16)
    x0e8_d = din("x0e8", [128, Tpc], FP8)
    rbf8_d = din("rbf8", [NRBF, Tpc], FP8)
    sbf8_d = din("sbf8", [NSBF, Tpc], FP8)
    dl_d = din("dl", [128, NB], F32)
    iota_d = din("iota", [128, 128], BF16)
    wkj_d = din("wkj", [128, 128], BF16)
    wrbf_d = din("wrbf", [NRBF, 128], BF16)
    wdown_d = din("wdown", [128, INT], BF16)
    wsbf_d = din("wsbf", [NSBF, INT], BF16)
    ident_d = din("ident", [128, 128], BF16)
    wji_d = din("wji", [128, 128], BF16)
    wup_d = din("wup", [INT, 128], BF16)
    wb1_d = din("wb1", [128, 128], BF16)
    wb2_d = din("wb2", [128, 128], BF16)
    wfin_d = din("wfin", [128, 128], BF16)
    wa11_d = din("wa11", [128, 128], BF16)
    wa12_d = din("wa12", [128, 128], BF16)
    wa21_d = din("wa21", [128, 128], BF16)
    wa22_d = din("wa22", [128, 128], BF16)
    bias_d = din("bias", [128, 8], F32)
    bias2_d = din("bias2", [128, 1], F32)
    outT_d = nc.dram_tensor("outT", [128, ECa], BF16, kind="ExternalOutput").ap()

    with tile.TileContext(nc) as tc:
        with (
            tc.tile_pool(name="const", bufs=1) as cp,
            tc.tile_pool(name="tbig", bufs=2) as tb,
            tc.tile_pool(name="small", bufs=4) as sp,
            tc.tile_pool(name="edge", bufs=2) as ep,
            tc.tile_pool(name="pbig", bufs=2, space="PSUM") as pb,
            tc.tile_pool(name="ptr", bufs=1, space="PSUM") as ptp,
            tc.tile_pool(name="pzs", bufs=2, space="PSUM") as pz,
            tc.tile_pool(name="pacc", bufs=1, space="PSUM") as pacc,
        ):
            def cload(d, shape, dt, tag):
                t = cp.tile(shape, dt, tag=tag, name=tag)
                nc.sync.dma_start(out=t[:], in_=d[:, :])
                return t
            iota_c = cload(iota_d, [128, 128], BF16, "iota")
            wkj_c = cload(wkj_d, [128, 128], BF16, "wkj")
            wrbf_c = cload(wrbf_d, [NRBF, 128], BF16, "wrbf")
            wdown_c = cload(wdown_d, [128, INT], BF16, "wdown")
            wsbf_c = cload(wsbf_d, [NSBF, INT], BF16, "wsbf")
            ident_c = cload(ident_d, [128, 128], BF16, "ident")
            wji_c = cload(wji_d, [128, 128], BF16, "wji")
            wup_c = cload(wup_d, [INT, 128], BF16, "wup")
            wb1_c = cload(wb1_d, [128, 128], BF16, "wb1")
            wb2_c = cload(wb2_d, [128, 128], BF16, "wb2")
            wfin_c = cload(wfin_d, [128, 128], BF16, "wfin")
            wa11_c = cload(wa11_d, [128, 128], BF16, "wa11")
            wa12_c = cload(wa12_d, [128, 128], BF16, "wa12")
            wa21_c = cload(wa21_d, [128, 128], BF16, "wa21")
            wa22_c = cload(wa22_d, [128, 128], BF16, "wa22")
            bias_c = cload(bias_d, [128, 8], F32, "bias")
            bias2_c = cload(bias2_d, [128, 1], F32, "bias2")
            bkj = bias_c[:, 0:1]; bji = bias_c[:, 1:2]
            bb1 = bias_c[:, 2:3]; bb2 = bias_c[:, 3:4]; bfin = bias_c[:, 4:5]
            ba11 = bias_c[:, 5:6]; ba12 = bias_c[:, 6:7]; ba21 = bias_c[:, 7:8]
            ba22 = bias2_c[:, 0:1]
            # x_kj segment sums stay in SBUF between the two phases
            xkj_all = cp.tile([128, NWC * INT], BF16, tag="xkj_all",
                              name="xkj_all")

            def body_t(i):
                x0e_t = tb.tile([128, SLOT], FP8, tag="x0e", name="x0e")
                nc.sync.dma_start(out=x0e_t[:], in_=x0e8_d[:, ts(i, SLOT)])
                rbf_t = tb.tile([NRBF, SLOT], FP8, tag="rbfe", name="rbfe")
                nc.sync.dma_start(out=rbf_t[:], in_=rbf8_d[:, ts(i, SLOT)])
                sbf_t = tb.tile([NSBF, SLOT], FP8, tag="sbf", name="sbf")
                nc.sync.dma_start(out=sbf_t[:], in_=sbf8_d[:, ts(i, SLOT)])
                dl_t = sp.tile([128, BPI], F32, tag="dl", name="dl")
                nc.sync.dma_start(out=dl_t[:], in_=dl_d[:, ts(i, BPI)])

                t3 = tb.tile([128, SLOT], BF16, tag="t3", name="t3")
                for j in range(NSUB):
                    sl = slice(j * 512, (j + 1) * 512)
                    pk = pb.tile([128, 512], F32, tag="pbig", name="pk")
                    nc.tensor.matmul(out=pk[:], lhsT=wkj_c[:], rhs=x0e_t[:, sl],
                                     start=True, stop=True)
                    t1 = sp.tile([128, 512], BF16, tag="t1", name="t1")
                    nc.scalar.activation(out=t1[:], in_=pk[:], func=act,
                                         bias=bkj)
                    pr = pb.tile([128, 512], F32, tag="pbig", name="pr")
                    nc.tensor.matmul(out=pr[:], lhsT=wrbf_c[:], rhs=rbf_t[:, sl],
                                     start=True, stop=True)
                    nc.vector.tensor_tensor(out=t3[:, sl], in0=t1[:], in1=pr[:],
                                            op=AOP.mult)

                y_t = sp.tile([128, BPI * INT], BF16, tag="y", name="y")
                for g in range(NGRP):
                    pzs = pz.tile([128, 2 * GRP * INT], F32, tag="pzs",
                                  name="pzs")
                    for k in range(GRP):
                        b = g * GRP + k
                        cb = slice(b * 128, (b + 1) * 128)
                        nc.tensor.matmul(out=pzs[:, k * INT:(k + 1) * INT],
                                         lhsT=t3[:, cb], rhs=wdown_c[:],
                                         start=True, stop=True)
                        nc.tensor.matmul(
                            out=pzs[:, (GRP + k) * INT:(GRP + k + 1) * INT],
                            lhsT=sbf_t[:, cb], rhs=wsbf_c[:],
                            start=True, stop=True)
                    zg = sp.tile([128, GRP * INT], BF16, tag="zb", name="zg")
                    nc.scalar.activation(out=zg[:], in_=pzs[:, :GRP * INT],
                                         func=act)
                    nc.vector.tensor_tensor(
                        out=y_t[:, g * GRP * INT:(g + 1) * GRP * INT],
                        in0=zg[:], in1=pzs[:, GRP * INT:], op=AOP.mult)

                acc = pacc.tile([128, WPS * INT], F32, tag="acc", name="acc")
                for w in range(WPS):
                    for k in range(NBW):
                        b = w * NBW + k
                        oh = sp.tile([128, 128], BF16, tag="oh", name="oh")
                        nc.vector.tensor_scalar(out=oh[:], in0=iota_c[:],
                                                scalar1=dl_t[:, b:b + 1],
                                                scalar2=None, op0=AOP.is_equal)
                        nc.tensor.matmul(out=acc[:, w * INT:(w + 1) * INT],
                                         lhsT=oh[:],
                                         rhs=y_t[:, b * INT:(b + 1) * INT],
                                         start=(k == 0), stop=(k == NBW - 1))
                nc.vector.tensor_copy(out=xkj_all[:, ts(i, WPS * INT)],
                                      in_=acc[:])

            def body_e(i):
                def each(fn):
                    for n in range(ew):
                        fn(n)

                x0_t = {}
                for n in range(ew):
                    x0_t[n] = ep.tile([128, EDG], BF16, tag=f"x0_{n}",
                                      name=f"x0_{n}")
                    nc.sync.dma_start(out=x0_t[n][:],
                                      in_=x0T_d[:, ts(i * ew + n, EDG)])

                pjis = {}
                def _ji_mm(n):
                    pjis[n] = pb.tile([128, 512], F32, tag="pbig", name="pji")
                    nc.tensor.matmul(out=pjis[n][:, :EDG], lhsT=wji_c[:],
                                     rhs=x0_t[n][:], start=True, stop=True)
                each(_ji_mm)
                x_ji = {}
                def _ji_act(n):
                    x_ji[n] = ep.tile([128, EDG], BF16, tag=f"xji_{n}",
                                      name=f"xji_{n}")
                    nc.scalar.activation(out=x_ji[n][:], in_=pjis[n][:, :EDG],
                                         func=act, bias=bji)
                each(_ji_act)
                ptrs = {}
                def _tr(n):
                    ptrs[n] = ptp.tile([128, 512], BF16, tag="ptrb", name="ptr")
                    for w in range(WPS):
                        nc.tensor.transpose(
                            out=ptrs[n][:INT, w * 128:(w + 1) * 128],
                            in_=xkj_all[:, ts((i * ew + n) * WPS + w, INT)],
                            identity=ident_c[:])
                each(_tr)
                xkjT = {}
                def _trc(n):
                    xkjT[n] = ep.tile([INT, EDG], BF16, tag=f"xkT_{n}",
                                      name=f"xkT_{n}")
                    nc.vector.tensor_copy(out=xkjT[n][:], in_=ptrs[n][:INT, :EDG])
                each(_trc)
                pus = {}
                def _up_mm(n):
                    pus[n] = pb.tile([128, 512], F32, tag="pbig", name="pup")
                    nc.tensor.matmul(out=pus[n][:, :EDG], lhsT=wup_c[:],
                                     rhs=xkjT[n][:], start=True, stop=True)
                each(_up_mm)
                x2 = {}
                def _up_act(n):
                    xe = ep.tile([128, EDG], BF16, tag=f"xke_{n}", name=f"xke_{n}")
                    nc.scalar.activation(out=xe[:], in_=pus[n][:, :EDG], func=act)
                    x2[n] = ep.tile([128, EDG], BF16, tag=f"x2_{n}",
                                    name=f"x2_{n}")
                    nc.vector.tensor_tensor(out=x2[n][:], in0=x_ji[n][:],
                                            in1=xe[:], op=AOP.add)
                each(_up_act)

                def dense(w_c, srcs, bias_ap, tag):
                    outs = {}
                    ps_ = {}
                    def _mm(n):
                        ps_[n] = pb.tile([128, 512], F32, tag="pbig", name="pd")
                        nc.tensor.matmul(out=ps_[n][:, :EDG], lhsT=w_c[:],
                                         rhs=srcs[n][:], start=True, stop=True)
                    each(_mm)
                    def _act(n):
                        outs[n] = ep.tile([128, EDG], BF16, tag=f"{tag}_{n}",
                                          name=f"{tag}_{n}")
                        nc.scalar.activation(out=outs[n][:], in_=ps_[n][:, :EDG],
                                             func=act, bias=bias_ap)
                    each(_act)
                    return outs

                h1 = dense(wb1_c, x2, bb1, "h")
                h2 = dense(wb2_c, h1, bb2, "i2")
                x2b = {}
                def _add1(n):
                    x2b[n] = ep.tile([128, EDG], BF16, tag=f"x2b_{n}",
                                     name=f"x2b_{n}")
                    nc.vector.tensor_tensor(out=x2b[n][:], in0=x2[n][:],
                                            in1=h2[n][:], op=AOP.add)
                each(_add1)
                x2f = dense(wfin_c, x2b, bfin, "j2")
                x = {}
                def _skip(n):
                    x[n] = ep.tile([128, EDG], BF16, tag=f"x_{n}", name=f"x_{n}")
                    nc.gpsimd.tensor_tensor(out=x[n][:], in0=x0_t[n][:],
                                            in1=x2f[n][:], op=AOP.add)
                each(_skip)
                g1 = dense(wa11_c, x, ba11, "h")
                g2 = dense(wa12_c, g1, ba12, "i2")
                xa = {}
                def _add2(n):
                    xa[n] = ep.tile([128, EDG], BF16, tag=f"xa_{n}",
                                    name=f"xa_{n}")
                    nc.gpsimd.tensor_tensor(out=xa[n][:], in0=x[n][:],
                                            in1=g2[n][:], op=AOP.add)
                each(_add2)
                g3 = dense(wa21_c, xa, ba21, "h")
                g4 = dense(wa22_c, g3, ba22, "i2")
                def _fin(n):
                    xf = ep.tile([128, EDG], BF16, tag=f"xf_{n}", name=f"xf_{n}")
                    nc.gpsimd.tensor_tensor(out=xf[:], in0=xa[n][:],
                                            in1=g4[n][:], op=AOP.add)
                    nc.sync.dma_start(out=outT_d[:, ts(i * ew + n, EDG)],
                                      in_=xf[:])
                each(_fin)

            def run_loop(fn, n_iter):
                if n_iter == 1:
                    fn(0)
                else:
                    tc.For_i_unrolled(0, n_iter, 1, fn,
                                      max_unroll=min(unroll, n_iter))

            def run_all():
                run_loop(body_t, SC)
                # full python unroll: transpose reads of xkj_all need
                # compile-time offsets (no register offsets in ldweights)
                for j in range(SCE):
                    body_e(j)

            if repeats > 1:
                with tc.For_i(0, repeats) as r:
                    run_all()
            else:
                run_all()


# ----------------------------------------------------------------------------
# entry point
# ----------------------------------------------------------------------------

def kernel(**inputs):
    import sys
    if '/opt/trn_rl_repo' not in sys.path:
        sys.path.insert(0, '/opt/trn_rl_repo')
    from concourse import bacc
    from concourse.bass_utils import run_bass_kernel_spmd

    np_inputs = {k: np.asarray(v) for k, v in inputs.items()}
    x0 = np.asarray(np_inputs["x0"], np.float32)
    rbf = np.asarray(np_inputs["rbf"], np.float32)
    sbf = np.asarray(np_inputs["sbf"], np.float32)
    ide = np.asarray(np_inputs["id_expand_kj"], np.int64)
    idr = np.asarray(np_inputs["id_reduce_ji"], np.int64)

    cores, NBW, perm = _prep_core_arrays(x0, rbf, sbf, ide, idr)
    weights = _prep_weights(np_inputs)

    key = ("v3", NBW)
    if key not in _CACHE:
        nc = bacc.Bacc("TRN2", target_bir_lowering=False, debug=False,
                       enable_asserts=True, num_devices=NCORES)
        build_program(nc, ECa=EC, NBW=NBW, WPS=4, repeats=1, unroll=8, ew=2)
        nc.compile()
        _CACHE[key] = nc
    nc = _CACHE[key]

    in_maps = []
    for c in range(NCORES):
        m = dict(cores[c])
        m.update(weights)
        in_maps.append(m)
    res = run_bass_kernel_spmd(nc, in_maps, core_ids=list(range(NCORES)))
    outp = np.concatenate(
        [res.results[c]["outT"].T.astype(np.float32) for c in range(NCORES)],
        axis=0)
    out = np.empty((E, EMB), np.float32)
    out[perm] = outp
    return out


# revision 22
# speedup vs baseline: 2.8814x; 1.0210x over previous
"""Trainium2 Bass kernel for the DimeNet++-style EquivariantInteractionPPBlock.

Strategy (8 NeuronCores, SPMD, no cross-core collectives):
  Triplets are routed on the host to the core owning their id_reduce_ji edge.
  Edges are permuted (host-side bin packing on triplet degree) so that every
  128-edge window receives exactly 1024 triplets -> zero padding and a fixed
  8-block segment-sum schedule per window.  Per-triplet inputs (x0/rbf gathered
  by id_expand_kj, plus sbf) are shipped as fp8-e4m3 streams (validated to be
  within tolerance with bf16 weights); all model math runs on device.  The
  segment-sum is PSUM-accumulated one-hot matmuls; the one-hot comes from an
  iota/is_equal against a host-provided window-local offset.  The summed
  per-window x_kj lives in SBUF between the triplet and edge phases (no DRAM
  round trip).  The per-edge dense stack runs in a second loop, with
  element-wise work split across the Vector and GpSimd engines.

kernel(**inputs) -> np.ndarray [E, 128] float32.
"""
import numpy as np
import ml_dtypes

BF16_NP = ml_dtypes.bfloat16
FP8_NP = ml_dtypes.float8_e4m3
E, T, EMB, INT, NRBF, NSBF = 262144, 2097152, 128, 64, 6, 42
NCORES = 8
EC = E // NCORES
WIN = 128
NWIN = E // WIN          # 2048 global windows
CAP = T // NWIN          # 1024 triplets per window when balanced

_CACHE = {}


# ----------------------------------------------------------------------------
# host-side edge balancing + input routing
# ----------------------------------------------------------------------------

def _balance_edges(deg):
    """Partition edges into NWIN windows of WIN edges with per-window triplet
    degree sums as close to CAP as possible (exactly CAP when achievable).
    Returns wins [NWIN, WIN] edge ids."""
    order = np.argsort(-deg, kind="stable").astype(np.int64)
    mat = order.reshape(WIN, NWIN).copy()
    mat[1::2] = mat[1::2, ::-1]          # serpentine deal
    wins = np.ascontiguousarray(mat.T)   # [NWIN, WIN]
    sums = deg[wins].sum(1)
    # repair: swap single edges between max/min windows until all <= CAP
    for _ in range(20000):
        o = int(np.argmax(sums))
        if sums[o] <= CAP:
            break
        u = int(np.argmin(sums))
        excess = min(sums[o] - CAP, CAP - sums[u])
        do, du = deg[wins[o]], deg[wins[u]]
        diff = do[:, None] - du[None, :]        # [WIN, WIN]
        good = diff[(diff > 0) & (diff <= excess)]
        target = good.max() if good.size else diff[diff > 0].min()
        i, j = np.argwhere(diff == target)[0]
        wins[o, i], wins[u, j] = wins[u, j], wins[o, i]
        d = int(target)
        sums[o] -= d
        sums[u] += d
    return wins, int(sums.max())


def _prep_core_arrays(x0, rbf, sbf, ide, idr):
    deg = np.bincount(idr, minlength=E)
    wins, mx = _balance_edges(deg)
    NBW = max(1, int(np.ceil(mx / 128)))

    winof = np.empty(E, np.int64)
    offof = np.empty(E, np.int64)
    winof[wins] = np.arange(NWIN)[:, None]
    offof[wins] = np.arange(WIN)[None, :]

    wt = winof[idr]                       # triplet -> target window
    sort_idx = np.argsort(wt, kind="stable")
    wt_s = wt[sort_idx]
    counts = np.bincount(wt, minlength=NWIN)
    starts = np.concatenate([[0], np.cumsum(counts)[:-1]])
    within = np.arange(T, dtype=np.int64) - starts[wt_s]
    wincap = NBW * 128
    slots = wt_s * wincap + within        # global padded slot id
    Tglob = NWIN * wincap
    Tpc = Tglob // NCORES

    ide_s = ide[sort_idx]
    if Tglob == T:
        # exact balance: slots == arange(T); direct layout, no scatter
        x0e8 = np.ascontiguousarray(x0[ide_s].T).astype(FP8_NP)
        rbf8 = np.ascontiguousarray(rbf[ide_s].T).astype(FP8_NP)
        sbf8 = np.ascontiguousarray(sbf[sort_idx].T).astype(FP8_NP)
        dl = offof[idr[sort_idx]].astype(np.float32)
    else:
        x0e8 = np.zeros((EMB, Tglob), FP8_NP)
        x0e8[:, slots] = x0[ide_s].T.astype(FP8_NP)
        rbf8 = np.zeros((NRBF, Tglob), FP8_NP)
        rbf8[:, slots] = rbf[ide_s].T.astype(FP8_NP)
        sbf8 = np.zeros((NSBF, Tglob), FP8_NP)
        sbf8[:, slots] = sbf[sort_idx].T.astype(FP8_NP)
        dl = np.full(Tglob, -1.0, np.float32)
        dl[slots] = offof[idr[sort_idx]].astype(np.float32)

    NBpc = Tpc // 128
    dl_t = np.ascontiguousarray(dl.reshape(NWIN * NBW, 128).T)  # [128, NB]

    cores = []
    for c in range(NCORES):
        edges_c = wins[c * (NWIN // NCORES):(c + 1) * (NWIN // NCORES)].ravel()
        sl = slice(c * Tpc, (c + 1) * Tpc)
        cores.append(dict(
            x0e8=np.ascontiguousarray(x0e8[:, sl]),
            rbf8=np.ascontiguousarray(rbf8[:, sl]),
            sbf8=np.ascontiguousarray(sbf8[:, sl]),
            dl=np.ascontiguousarray(dl_t[:, c * NBpc:(c + 1) * NBpc]),
            x0T=np.ascontiguousarray(x0[edges_c].T).astype(BF16_NP),
        ))
    perm = wins.ravel()                   # output row order
    return cores, NBW, perm


def _prep_weights(inputs):
    f32 = np.float32
    Wrbf = (np.asarray(inputs["w_rbf1"], f32) @ np.asarray(inputs["w_rbf2"], f32))
    Wsbf = (np.asarray(inputs["w_sbf1"], f32) @ np.asarray(inputs["w_sbf2"], f32))
    iota = np.tile(np.arange(128, dtype=f32)[None, :], (128, 1)).astype(BF16_NP)
    ident = np.eye(128, dtype=f32)
    bias = np.zeros((128, 8), f32)
    for col, key in enumerate(["b_kj", "b_ji", "bb1", "bb2", "b_fin"]):
        bias[:, col] = np.asarray(inputs[key], f32)
    bias[:, 5] = np.asarray(inputs["ba1"][0], f32)
    bias[:, 6] = np.asarray(inputs["ba2"][0], f32)
    bias[:, 7] = np.asarray(inputs["ba1"][1], f32)
    bias2 = np.asarray(inputs["ba2"][1], f32).reshape(128, 1)
    b16 = lambda a: np.asarray(a, f32).astype(BF16_NP)
    return dict(
        iota=iota,
        wkj=b16(inputs["w_kj"]), wrbf=Wrbf.astype(BF16_NP),
        wdown=b16(inputs["w_down"]), wsbf=Wsbf.astype(BF16_NP),
        ident=ident.astype(BF16_NP),
        wji=b16(inputs["w_ji"]), wup=b16(inputs["w_up"]),
        wb1=b16(inputs["wb1"]), wb2=b16(inputs["wb2"]), wfin=b16(inputs["w_fin"]),
        wa11=b16(inputs["wa1"][0]), wa12=b16(inputs["wa2"][0]),
        wa21=b16(inputs["wa1"][1]), wa22=b16(inputs["wa2"][1]),
        bias=bias, bias2=bias2,
    )


# ----------------------------------------------------------------------------
# bass program
# ----------------------------------------------------------------------------

def build_program(nc, ECa, NBW, WPS=4, repeats=1, unroll=8, act=None, ew=2):
    from concourse import mybir, tile
    from concourse.bass import ts
    F32 = mybir.dt.float32
    BF16 = mybir.dt.bfloat16
    FP8 = mybir.dt.float8e4
    AOP = mybir.AluOpType
    ACTF = mybir.ActivationFunctionType
    if act is None:
        act = ACTF.Silu

    NWC = ECa // 128
    SC = NWC // WPS
    NB = NWC * NBW
    Tpc = NB * 128
    SLOT = WPS * NBW * 128
    EDG = WPS * 128
    BPI = WPS * NBW
    NSUB = SLOT // 512
    SCE = SC // ew
    GRP = 8 if BPI % 8 == 0 else (4 if BPI % 4 == 0 else (3 if BPI % 3 == 0 else 1))
    assert BPI % GRP == 0
    NGRP = BPI // GRP

    def din(name, shape, dt):
        return nc.dram_tensor(name, shape, dt, kind="ExternalInput").ap()

    x0T_d = din("x0T", [128, ECa], BF16)
    x0e8_d = din("x0e8", [128, Tpc], FP8)
    rbf8_d = din("rbf8", [NRBF, Tpc], FP8)
    sbf8_d = din("sbf8", [NSBF, Tpc], FP8)
    dl_d = din("dl", [128, NB], F32)
    iota_d = din("iota", [128, 128], BF16)
    wkj_d = din("wkj", [128, 128], BF16)
    wrbf_d = din("wrbf", [NRBF, 128], BF16)
    wdown_d = din("wdown", [128, INT], BF16)
    wsbf_d = din("wsbf", [NSBF, INT], BF16)
    ident_d = din("ident", [128, 128], BF16)
    wji_d = din("wji", [128, 128], BF16)
    wup_d = din("wup", [INT, 128], BF16)
    wb1_d = din("wb1", [128, 128], BF16)
    wb2_d = din("wb2", [128, 128], BF16)
    wfin_d = din("wfin", [128, 128], BF16)
    wa11_d = din("wa11", [128, 128], BF16)
    wa12_d = din("wa12", [128, 128], BF16)
    wa21_d = din("wa21", [128, 128], BF16)
    wa22_d = din("wa22", [128, 128], BF16)
    bias_d = din("bias", [128, 8], F32)
    bias2_d = din("bias2", [128, 1], F32)
    outT_d = nc.dram_tensor("outT", [128, ECa], BF16, kind="ExternalOutput").ap()

    with tile.TileContext(nc) as tc:
        with (
            tc.tile_pool(name="const", bufs=1) as cp,
            tc.tile_pool(name="tbig", bufs=2) as tb,
            tc.tile_pool(name="small", bufs=4) as sp,
            tc.tile_pool(name="edge", bufs=2) as ep,
            tc.tile_pool(name="pbig", bufs=2, space="PSUM") as pb,
            tc.tile_pool(name="ptr", bufs=1, space="PSUM") as ptp,
            tc.tile_pool(name="pzs", bufs=2, space="PSUM") as pz,
            tc.tile_pool(name="pacc", bufs=1, space="PSUM") as pacc,
        ):
            def cload(d, shape, dt, tag):
                t = cp.tile(shape, dt, tag=tag, name=tag)
                nc.sync.dma_start(out=t[:], in_=d[:, :])
                return t
            iota_c = cload(iota_d, [128, 128], BF16, "iota")
            wkj_c = cload(wkj_d, [128, 128], BF16, "wkj")
            wrbf_c = cload(wrbf_d, [NRBF, 128], BF16, "wrbf")
            wdown_c = cload(wdown_d, [128, INT], BF16, "wdown")
            wsbf_c = cload(wsbf_d, [NSBF, INT], BF16, "wsbf")
            ident_c = cload(ident_d, [128, 128], BF16, "ident")
            wji_c = cload(wji_d, [128, 128], BF16, "wji")
            wup_c = cload(wup_d, [INT, 128], BF16, "wup")
            wb1_c = cload(wb1_d, [128, 128], BF16, "wb1")
            wb2_c = cload(wb2_d, [128, 128], BF16, "wb2")
            wfin_c = cload(wfin_d, [128, 128], BF16, "wfin")
            wa11_c = cload(wa11_d, [128, 128], BF16, "wa11")
            wa12_c = cload(wa12_d, [128, 128], BF16, "wa12")
            wa21_c = cload(wa21_d, [128, 128], BF16, "wa21")
            wa22_c = cload(wa22_d, [128, 128], BF16, "wa22")
            bias_c = cload(bias_d, [128, 8], F32, "bias")
            bias2_c = cload(bias2_d, [128, 1], F32, "bias2")
            bkj = bias_c[:, 0:1]; bji = bias_c[:, 1:2]
            bb1 = bias_c[:, 2:3]; bb2 = bias_c[:, 3:4]; bfin = bias_c[:, 4:5]
            ba11 = bias_c[:, 5:6]; ba12 = bias_c[:, 6:7]; ba21 = bias_c[:, 7:8]
            ba22 = bias2_c[:, 0:1]
            # x_kj segment sums stay in SBUF between the two phases
            xkj_all = cp.tile([128, NWC * INT], BF16, tag="xkj_all",
                              name="xkj_all")

            def body_t(i):
                x0e_t = tb.tile([128, SLOT], FP8, tag="x0e", name="x0e")
                nc.sync.dma_start(out=x0e_t[:], in_=x0e8_d[:, ts(i, SLOT)])
                rbf_t = tb.tile([NRBF, SLOT], FP8, tag="rbfe", name="rbfe")
                nc.sync.dma_start(out=rbf_t[:], in_=rbf8_d[:, ts(i, SLOT)])
                sbf_t = tb.tile([NSBF, SLOT], FP8, tag="sbf", name="sbf")
                nc.sync.dma_start(out=sbf_t[:], in_=sbf8_d[:, ts(i, SLOT)])
                dl_t = sp.tile([128, BPI], F32, tag="dl", name="dl")
                nc.sync.dma_start(out=dl_t[:], in_=dl_d[:, ts(i, BPI)])

                t3 = tb.tile([128, SLOT], BF16, tag="t3", name="t3")
                for j in range(NSUB):
                    sl = slice(j * 512, (j + 1) * 512)
                    pk = pb.tile([128, 512], F32, tag="pbig", name="pk")
                    nc.tensor.matmul(out=pk[:], lhsT=wkj_c[:], rhs=x0e_t[:, sl],
                                     start=True, stop=True)
                    t1 = sp.tile([128, 512], BF16, tag="t1", name="t1")
                    nc.scalar.activation(out=t1[:], in_=pk[:], func=act,
                                         bias=bkj)
                    pr = pb.tile([128, 512], F32, tag="pbig", name="pr")
                    nc.tensor.matmul(out=pr[:], lhsT=wrbf_c[:], rhs=rbf_t[:, sl],
                                     start=True, stop=True)
                    nc.vector.tensor_tensor(out=t3[:, sl], in0=t1[:], in1=pr[:],
                                            op=AOP.mult)

                y_t = sp.tile([128, BPI * INT], BF16, tag="y", name="y")
                for g in range(NGRP):
                    pzs = pz.tile([128, 2 * GRP * INT], F32, tag="pzs",
                                  name="pzs")
                    for k in range(GRP):
                        b = g * GRP + k
                        cb = slice(b * 128, (b + 1) * 128)
                        nc.tensor.matmul(out=pzs[:, k * INT:(k + 1) * INT],
                                         lhsT=t3[:, cb], rhs=wdown_c[:],
                                         start=True, stop=True)
                        nc.tensor.matmul(
                            out=pzs[:, (GRP + k) * INT:(GRP + k + 1) * INT],
                            lhsT=sbf_t[:, cb], rhs=wsbf_c[:],
                            start=True, stop=True)
                    zg = sp.tile([128, GRP * INT], BF16, tag="zb", name="zg")
                    nc.scalar.activation(out=zg[:], in_=pzs[:, :GRP * INT],
                                         func=act)
                    nc.vector.tensor_tensor(
                        out=y_t[:, g * GRP * INT:(g + 1) * GRP * INT],
                        in0=zg[:], in1=pzs[:, GRP * INT:], op=AOP.mult)

                acc = pacc.tile([128, WPS * INT], F32, tag="acc", name="acc")
                for w in range(WPS):
                    for k in range(NBW):
                        b = w * NBW + k
                        oh = sp.tile([128, 128], BF16, tag="oh", name="oh")
                        nc.vector.tensor_scalar(out=oh[:], in0=iota_c[:],
                                                scalar1=dl_t[:, b:b + 1],
                                                scalar2=None, op0=AOP.is_equal)
                        nc.tensor.matmul(out=acc[:, w * INT:(w + 1) * INT],
                                         lhsT=oh[:],
                                         rhs=y_t[:, b * INT:(b + 1) * INT],
                                         start=(k == 0), stop=(k == NBW - 1))
                nc.vector.tensor_copy(out=xkj_all[:, ts(i, WPS * INT)],
                                      in_=acc[:])

            def body_e(i):
                def each(fn):
                    for n in range(ew):
                        fn(n)

                x0_t = {}
                for n in range(ew):
                    x0_t[n] = ep.tile([128, EDG], BF16, tag=f"x0_{n}",
                                      name=f"x0_{n}")
                    nc.sync.dma_start(out=x0_t[n][:],
                                      in_=x0T_d[:, ts(i * ew + n, EDG)])

                pjis = {}
                def _ji_mm(n):
                    pjis[n] = pb.tile([128, 512], F32, tag="pbig", name="pji")
                    nc.tensor.matmul(out=pjis[n][:, :EDG], lhsT=wji_c[:],
                                     rhs=x0_t[n][:], start=True, stop=True)
                each(_ji_mm)
                x_ji = {}
                def _ji_act(n):
                    x_ji[n] = ep.tile([128, EDG], BF16, tag=f"xji_{n}",
                                      name=f"xji_{n}")
                    nc.scalar.activation(out=x_ji[n][:], in_=pjis[n][:, :EDG],
                                         func=act, bias=bji)
                each(_ji_act)
                ptrs = {}
                def _tr(n):
                    ptrs[n] = ptp.tile([128, 512], BF16, tag="ptrb", name="ptr")
                    for w in range(WPS):
                        nc.tensor.transpose(
                            out=ptrs[n][:INT, w * 128:(w + 1) * 128],
                            in_=xkj_all[:, ts((i * ew + n) * WPS + w, INT)],
                            identity=ident_c[:])
                each(_tr)
                xkjT = {}
                def _trc(n):
                    xkjT[n] = ep.tile([INT, EDG], BF16, tag=f"xkT_{n}",
                                      name=f"xkT_{n}")
                    nc.vector.tensor_copy(out=xkjT[n][:], in_=ptrs[n][:INT, :EDG])
                each(_trc)
                pus = {}
                def _up_mm(n):
                    pus[n] = pb.tile([128, 512], F32, tag="pbig", name="pup")
                    nc.tensor.matmul(out=pus[n][:, :EDG], lhsT=wup_c[:],
                                     rhs=xkjT[n][:], start=True, stop=True)
                each(_up_mm)
                x2 = {}
                def _up_act(n):
                    xe = ep.tile([128, EDG], BF16, tag=f"xke_{n}", name=f"xke_{n}")
                    nc.scalar.activation(out=xe[:], in_=pus[n][:, :EDG], func=act)
                    x2[n] = ep.tile([128, EDG], BF16, tag=f"x2_{n}",
                                    name=f"x2_{n}")
                    nc.vector.tensor_tensor(out=x2[n][:], in0=x_ji[n][:],
                                            in1=xe[:], op=AOP.add)
                each(_up_act)

                def dense(w_c, srcs, bias_ap, tag):
                    outs = {}
                    ps_ = {}
                    def _mm(n):
                        ps_[n] = pb.tile([128, 512], F32, tag="pbig", name="pd")
                        nc.tensor.matmul(out=ps_[n][:, :EDG], lhsT=w_c[:],
                                         rhs=srcs[n][:], start=True, stop=True)
                    each(_mm)
                    def _act(n):
                        outs[n] = ep.tile([128, EDG], BF16, tag=f"{tag}_{n}",
                                          name=f"{tag}_{n}")
                        nc.scalar.activation(out=outs[n][:], in_=ps_[n][:, :EDG],
                                             func=act, bias=bias_ap)
                    each(_act)
                    return outs

                h1 = dense(wb1_c, x2, bb1, "h")
                h2 = dense(wb2_c, h1, bb2, "i2")
                x2b = {}
                def _add1(n):
                    x2b[n] = ep.tile([128, EDG], BF16, tag=f"x2b_{n}",
                                     name=f"x2b_{n}")
                    nc.vector.tensor_tensor(out=x2b[n][:], in0=x2[n][:],
                                            in1=h2[n][:], op=AOP.add)
                each(_add1)
                x2f = dense(wfin_c, x2b, bfin, "j2")
                x = {}
                def _skip(n):
                    x[n] = ep.tile([128, EDG], BF16, tag=f"x_{n}", name=f"x_{n}")
                    nc.gpsimd.tensor_tensor(out=x[n][:], in0=x0_t[n][:],
                                            in1=x2f[n][:], op=AOP.add)
                each(_skip)
                g1 = dense(wa11_c, x, ba11, "h")
                g2 = dense(wa12_c, g1, ba12, "i2")
                xa = {}
                def _add2(n):
                    xa[n] = ep.tile([128, EDG], BF16, tag=f"xa_{n}",
                                    name=f"xa_{n}")
                    nc.gpsimd.tensor_tensor(out=xa[n][:], in0=x[n][:],
                                            in1=g2[n][:], op=AOP.add)
                each(_add2)
                g3 = dense(wa21_c, xa, ba21, "h")
                g4 = dense(wa22_c, g3, ba22, "i2")
                def _fin(n):
                    xf = ep.tile([128, EDG], BF16, tag=f"xf_{n}", name=f"xf_{n}")
                    nc.gpsimd.tensor_tensor(out=xf[:], in0=xa[n][:],
                                            in1=g4[n][:], op=AOP.add)
                    nc.sync.dma_start(out=outT_d[:, ts(i * ew + n, EDG)],
                                      in_=xf[:])
                each(_fin)

            def run_loop(fn, n_iter):
                if n_iter == 1:
                    fn(0)
                else:
                    tc.For_i_unrolled(0, n_iter, 1, fn,
                                      max_unroll=min(unroll, n_iter))

            def run_all():
                run_loop(body_t, SC)
                # full python unroll: transpose reads of xkj_all need
                # compile-time offsets (no register offsets in ldweights)
                for j in range(SCE):
                    body_e(j)

            if repeats > 1:
                with tc.For_i(0, repeats) as r:
                    run_all()
            else:
                run_all()


# ----------------------------------------------------------------------------
# entry point
# ----------------------------------------------------------------------------

def kernel(**inputs):
    import sys
    if '/opt/trn_rl_repo' not in sys.path:
        sys.path.insert(0, '/opt/trn_rl_repo')
    from concourse import bacc
    from concourse.bass_utils import run_bass_kernel_spmd

    np_inputs = {k: np.asarray(v) for k, v in inputs.items()}
    x0 = np.asarray(np_inputs["x0"], np.float32)
    rbf = np.asarray(np_inputs["rbf"], np.float32)
    sbf = np.asarray(np_inputs["sbf"], np.float32)
    ide = np.asarray(np_inputs["id_expand_kj"], np.int64)
    idr = np.asarray(np_inputs["id_reduce_ji"], np.int64)

    cores, NBW, perm = _prep_core_arrays(x0, rbf, sbf, ide, idr)
    weights = _prep_weights(np_inputs)

    key = ("v3", NBW)
    if key not in _CACHE:
        nc = bacc.Bacc("TRN2", target_bir_lowering=False, debug=False,
                       enable_asserts=True, num_devices=NCORES)
        build_program(nc, ECa=EC, NBW=NBW, WPS=4, repeats=1, unroll=16, ew=2)
        nc.compile()
        _CACHE[key] = nc
    nc = _CACHE[key]

    in_maps = []
    for c in range(NCORES):
        m = dict(cores[c])
        m.update(weights)
        in_maps.append(m)
    res = run_bass_kernel_spmd(nc, in_maps, core_ids=list(range(NCORES)))
    outp = np.concatenate(
        [res.results[c]["outT"].T.astype(np.float32) for c in range(NCORES)],
        axis=0)
    out = np.empty((E, EMB), np.float32)
    out[perm] = outp
    return out


# revision 23
# speedup vs baseline: 2.9280x; 1.0162x over previous
"""Trainium2 Bass kernel for the DimeNet++-style EquivariantInteractionPPBlock.

Strategy (8 NeuronCores, SPMD, no cross-core collectives):
  Triplets are routed on the host to the core owning their id_reduce_ji edge.
  Edges are permuted (host-side bin packing on triplet degree) so that every
  128-edge window receives exactly 1024 triplets -> zero padding and a fixed
  8-block segment-sum schedule per window.  Per-triplet inputs (x0/rbf gathered
  by id_expand_kj, plus sbf) are shipped as fp8-e4m3 streams (validated to be
  within tolerance with bf16 weights); all model math runs on device.  The
  segment-sum is PSUM-accumulated one-hot matmuls; the one-hot comes from an
  iota/is_equal against a host-provided window-local offset.  The summed
  per-window x_kj lives in SBUF between the triplet and edge phases (no DRAM
  round trip).  The per-edge dense stack runs in a second loop, with
  element-wise work split across the Vector and GpSimd engines.

kernel(**inputs) -> np.ndarray [E, 128] float32.
"""
import numpy as np
import ml_dtypes

BF16_NP = ml_dtypes.bfloat16
FP8_NP = ml_dtypes.float8_e4m3
E, T, EMB, INT, NRBF, NSBF = 262144, 2097152, 128, 64, 6, 42
NCORES = 8
EC = E // NCORES
WIN = 128
NWIN = E // WIN          # 2048 global windows
CAP = T // NWIN          # 1024 triplets per window when balanced

_CACHE = {}


# ----------------------------------------------------------------------------
# host-side edge balancing + input routing
# ----------------------------------------------------------------------------

def _balance_edges(deg):
    """Partition edges into NWIN windows of WIN edges with per-window triplet
    degree sums as close to CAP as possible (exactly CAP when achievable).
    Returns wins [NWIN, WIN] edge ids."""
    order = np.argsort(-deg, kind="stable").astype(np.int64)
    mat = order.reshape(WIN, NWIN).copy()
    mat[1::2] = mat[1::2, ::-1]          # serpentine deal
    wins = np.ascontiguousarray(mat.T)   # [NWIN, WIN]
    sums = deg[wins].sum(1)
    # repair: swap single edges between max/min windows until all <= CAP
    for _ in range(20000):
        o = int(np.argmax(sums))
        if sums[o] <= CAP:
            break
        u = int(np.argmin(sums))
        excess = min(sums[o] - CAP, CAP - sums[u])
        do, du = deg[wins[o]], deg[wins[u]]
        diff = do[:, None] - du[None, :]        # [WIN, WIN]
        good = diff[(diff > 0) & (diff <= excess)]
        target = good.max() if good.size else diff[diff > 0].min()
        i, j = np.argwhere(diff == target)[0]
        wins[o, i], wins[u, j] = wins[u, j], wins[o, i]
        d = int(target)
        sums[o] -= d
        sums[u] += d
    return wins, int(sums.max())


def _prep_core_arrays(x0, rbf, sbf, ide, idr):
    deg = np.bincount(idr, minlength=E)
    wins, mx = _balance_edges(deg)
    NBW = max(1, int(np.ceil(mx / 128)))

    winof = np.empty(E, np.int64)
    offof = np.empty(E, np.int64)
    winof[wins] = np.arange(NWIN)[:, None]
    offof[wins] = np.arange(WIN)[None, :]

    wt = winof[idr]                       # triplet -> target window
    sort_idx = np.argsort(wt, kind="stable")
    wt_s = wt[sort_idx]
    counts = np.bincount(wt, minlength=NWIN)
    starts = np.concatenate([[0], np.cumsum(counts)[:-1]])
    within = np.arange(T, dtype=np.int64) - starts[wt_s]
    wincap = NBW * 128
    slots = wt_s * wincap + within        # global padded slot id
    Tglob = NWIN * wincap
    Tpc = Tglob // NCORES

    ide_s = ide[sort_idx]
    if Tglob == T:
        # exact balance: slots == arange(T); direct layout, no scatter
        x0e8 = np.ascontiguousarray(x0[ide_s].T).astype(FP8_NP)
        rbf8 = np.ascontiguousarray(rbf[ide_s].T).astype(FP8_NP)
        sbf8 = np.ascontiguousarray(sbf[sort_idx].T).astype(FP8_NP)
        dl = offof[idr[sort_idx]].astype(np.float32)
    else:
        x0e8 = np.zeros((EMB, Tglob), FP8_NP)
        x0e8[:, slots] = x0[ide_s].T.astype(FP8_NP)
        rbf8 = np.zeros((NRBF, Tglob), FP8_NP)
        rbf8[:, slots] = rbf[ide_s].T.astype(FP8_NP)
        sbf8 = np.zeros((NSBF, Tglob), FP8_NP)
        sbf8[:, slots] = sbf[sort_idx].T.astype(FP8_NP)
        dl = np.full(Tglob, -1.0, np.float32)
        dl[slots] = offof[idr[sort_idx]].astype(np.float32)

    NBpc = Tpc // 128
    dl_t = np.ascontiguousarray(dl.reshape(NWIN * NBW, 128).T)  # [128, NB]

    cores = []
    for c in range(NCORES):
        edges_c = wins[c * (NWIN // NCORES):(c + 1) * (NWIN // NCORES)].ravel()
        sl = slice(c * Tpc, (c + 1) * Tpc)
        cores.append(dict(
            x0e8=np.ascontiguousarray(x0e8[:, sl]),
            rbf8=np.ascontiguousarray(rbf8[:, sl]),
            sbf8=np.ascontiguousarray(sbf8[:, sl]),
            dl=np.ascontiguousarray(dl_t[:, c * NBpc:(c + 1) * NBpc]),
            x0T=np.ascontiguousarray(x0[edges_c].T).astype(BF16_NP),
        ))
    perm = wins.ravel()                   # output row order
    return cores, NBW, perm


def _prep_weights(inputs):
    f32 = np.float32
    Wrbf = (np.asarray(inputs["w_rbf1"], f32) @ np.asarray(inputs["w_rbf2"], f32))
    Wsbf = (np.asarray(inputs["w_sbf1"], f32) @ np.asarray(inputs["w_sbf2"], f32))
    iota = np.tile(np.arange(128, dtype=f32)[None, :], (128, 1)).astype(BF16_NP)
    ident = np.eye(128, dtype=f32)
    bias = np.zeros((128, 8), f32)
    for col, key in enumerate(["b_kj", "b_ji", "bb1", "bb2", "b_fin"]):
        bias[:, col] = np.asarray(inputs[key], f32)
    bias[:, 5] = np.asarray(inputs["ba1"][0], f32)
    bias[:, 6] = np.asarray(inputs["ba2"][0], f32)
    bias[:, 7] = np.asarray(inputs["ba1"][1], f32)
    bias2 = np.asarray(inputs["ba2"][1], f32).reshape(128, 1)
    b16 = lambda a: np.asarray(a, f32).astype(BF16_NP)
    return dict(
        iota=iota,
        wkj=b16(inputs["w_kj"]), wrbf=Wrbf.astype(BF16_NP),
        wdown=b16(inputs["w_down"]), wsbf=Wsbf.astype(BF16_NP),
        ident=ident.astype(BF16_NP),
        wji=b16(inputs["w_ji"]), wup=b16(inputs["w_up"]),
        wb1=b16(inputs["wb1"]), wb2=b16(inputs["wb2"]), wfin=b16(inputs["w_fin"]),
        wa11=b16(inputs["wa1"][0]), wa12=b16(inputs["wa2"][0]),
        wa21=b16(inputs["wa1"][1]), wa22=b16(inputs["wa2"][1]),
        bias=bias, bias2=bias2,
    )


# ----------------------------------------------------------------------------
# bass program
# ----------------------------------------------------------------------------

def build_program(nc, ECa, NBW, WPS=4, repeats=1, unroll=8, act=None, ew=2):
    from concourse import mybir, tile
    from concourse.bass import ts
    F32 = mybir.dt.float32
    BF16 = mybir.dt.bfloat16
    FP8 = mybir.dt.float8e4
    AOP = mybir.AluOpType
    ACTF = mybir.ActivationFunctionType
    if act is None:
        act = ACTF.Silu

    NWC = ECa // 128
    SC = NWC // WPS
    NB = NWC * NBW
    Tpc = NB * 128
    SLOT = WPS * NBW * 128
    EDG = WPS * 128
    BPI = WPS * NBW
    NSUB = SLOT // 512
    SCE = SC // ew
    GRP = 8 if BPI % 8 == 0 else (4 if BPI % 4 == 0 else (3 if BPI % 3 == 0 else 1))
    assert BPI % GRP == 0
    NGRP = BPI // GRP

    def din(name, shape, dt):
        return nc.dram_tensor(name, shape, dt, kind="ExternalInput").ap()

    x0T_d = din("x0T", [128, ECa], BF16)
    x0e8_d = din("x0e8", [128, Tpc], FP8)
    rbf8_d = din("rbf8", [NRBF, Tpc], FP8)
    sbf8_d = din("sbf8", [NSBF, Tpc], FP8)
    dl_d = din("dl", [128, NB], F32)
    iota_d = din("iota", [128, 128], BF16)
    wkj_d = din("wkj", [128, 128], BF16)
    wrbf_d = din("wrbf", [NRBF, 128], BF16)
    wdown_d = din("wdown", [128, INT], BF16)
    wsbf_d = din("wsbf", [NSBF, INT], BF16)
    ident_d = din("ident", [128, 128], BF16)
    wji_d = din("wji", [128, 128], BF16)
    wup_d = din("wup", [INT, 128], BF16)
    wb1_d = din("wb1", [128, 128], BF16)
    wb2_d = din("wb2", [128, 128], BF16)
    wfin_d = din("wfin", [128, 128], BF16)
    wa11_d = din("wa11", [128, 128], BF16)
    wa12_d = din("wa12", [128, 128], BF16)
    wa21_d = din("wa21", [128, 128], BF16)
    wa22_d = din("wa22", [128, 128], BF16)
    bias_d = din("bias", [128, 8], F32)
    bias2_d = din("bias2", [128, 1], F32)
    outT_d = nc.dram_tensor("outT", [128, ECa], BF16, kind="ExternalOutput").ap()

    with tile.TileContext(nc) as tc:
        with (
            tc.tile_pool(name="const", bufs=1) as cp,
            tc.tile_pool(name="tbig", bufs=2) as tb,
            tc.tile_pool(name="small", bufs=6) as sp,
            tc.tile_pool(name="edge", bufs=2) as ep,
            tc.tile_pool(name="pbig", bufs=2, space="PSUM") as pb,
            tc.tile_pool(name="ptr", bufs=1, space="PSUM") as ptp,
            tc.tile_pool(name="pzs", bufs=2, space="PSUM") as pz,
            tc.tile_pool(name="pacc", bufs=1, space="PSUM") as pacc,
        ):
            def cload(d, shape, dt, tag):
                t = cp.tile(shape, dt, tag=tag, name=tag)
                nc.sync.dma_start(out=t[:], in_=d[:, :])
                return t
            iota_c = cload(iota_d, [128, 128], BF16, "iota")
            wkj_c = cload(wkj_d, [128, 128], BF16, "wkj")
            wrbf_c = cload(wrbf_d, [NRBF, 128], BF16, "wrbf")
            wdown_c = cload(wdown_d, [128, INT], BF16, "wdown")
            wsbf_c = cload(wsbf_d, [NSBF, INT], BF16, "wsbf")
            ident_c = cload(ident_d, [128, 128], BF16, "ident")
            wji_c = cload(wji_d, [128, 128], BF16, "wji")
            wup_c = cload(wup_d, [INT, 128], BF16, "wup")
            wb1_c = cload(wb1_d, [128, 128], BF16, "wb1")
            wb2_c = cload(wb2_d, [128, 128], BF16, "wb2")
            wfin_c = cload(wfin_d, [128, 128], BF16, "wfin")
            wa11_c = cload(wa11_d, [128, 128], BF16, "wa11")
            wa12_c = cload(wa12_d, [128, 128], BF16, "wa12")
            wa21_c = cload(wa21_d, [128, 128], BF16, "wa21")
            wa22_c = cload(wa22_d, [128, 128], BF16, "wa22")
            bias_c = cload(bias_d, [128, 8], F32, "bias")
            bias2_c = cload(bias2_d, [128, 1], F32, "bias2")
            bkj = bias_c[:, 0:1]; bji = bias_c[:, 1:2]
            bb1 = bias_c[:, 2:3]; bb2 = bias_c[:, 3:4]; bfin = bias_c[:, 4:5]
            ba11 = bias_c[:, 5:6]; ba12 = bias_c[:, 6:7]; ba21 = bias_c[:, 7:8]
            ba22 = bias2_c[:, 0:1]
            # x_kj segment sums stay in SBUF between the two phases
            xkj_all = cp.tile([128, NWC * INT], BF16, tag="xkj_all",
                              name="xkj_all")

            def body_t(i):
                x0e_t = tb.tile([128, SLOT], FP8, tag="x0e", name="x0e")
                nc.sync.dma_start(out=x0e_t[:], in_=x0e8_d[:, ts(i, SLOT)])
                rbf_t = tb.tile([NRBF, SLOT], FP8, tag="rbfe", name="rbfe")
                nc.sync.dma_start(out=rbf_t[:], in_=rbf8_d[:, ts(i, SLOT)])
                sbf_t = tb.tile([NSBF, SLOT], FP8, tag="sbf", name="sbf")
                nc.sync.dma_start(out=sbf_t[:], in_=sbf8_d[:, ts(i, SLOT)])
                dl_t = sp.tile([128, BPI], F32, tag="dl", name="dl")
                nc.sync.dma_start(out=dl_t[:], in_=dl_d[:, ts(i, BPI)])

                t3 = tb.tile([128, SLOT], BF16, tag="t3", name="t3")
                for j in range(NSUB):
                    sl = slice(j * 512, (j + 1) * 512)
                    pk = pb.tile([128, 512], F32, tag="pbig", name="pk")
                    nc.tensor.matmul(out=pk[:], lhsT=wkj_c[:], rhs=x0e_t[:, sl],
                                     start=True, stop=True)
                    t1 = sp.tile([128, 512], BF16, tag="t1", name="t1")
                    nc.scalar.activation(out=t1[:], in_=pk[:], func=act,
                                         bias=bkj)
                    pr = pb.tile([128, 512], F32, tag="pbig", name="pr")
                    nc.tensor.matmul(out=pr[:], lhsT=wrbf_c[:], rhs=rbf_t[:, sl],
                                     start=True, stop=True)
                    nc.vector.tensor_tensor(out=t3[:, sl], in0=t1[:], in1=pr[:],
                                            op=AOP.mult)

                y_t = sp.tile([128, BPI * INT], BF16, tag="y", name="y")
                for g in range(NGRP):
                    pzs = pz.tile([128, 2 * GRP * INT], F32, tag="pzs",
                                  name="pzs")
                    for k in range(GRP):
                        b = g * GRP + k
                        cb = slice(b * 128, (b + 1) * 128)
                        nc.tensor.matmul(out=pzs[:, k * INT:(k + 1) * INT],
                                         lhsT=t3[:, cb], rhs=wdown_c[:],
                                         start=True, stop=True)
                        nc.tensor.matmul(
                            out=pzs[:, (GRP + k) * INT:(GRP + k + 1) * INT],
                            lhsT=sbf_t[:, cb], rhs=wsbf_c[:],
                            start=True, stop=True)
                    zg = sp.tile([128, GRP * INT], BF16, tag="zb", name="zg")
                    nc.scalar.activation(out=zg[:], in_=pzs[:, :GRP * INT],
                                         func=act)
                    nc.vector.tensor_tensor(
                        out=y_t[:, g * GRP * INT:(g + 1) * GRP * INT],
                        in0=zg[:], in1=pzs[:, GRP * INT:], op=AOP.mult)

                acc = pacc.tile([128, WPS * INT], F32, tag="acc", name="acc")
                for w in range(WPS):
                    for k in range(NBW):
                        b = w * NBW + k
                        oh = sp.tile([128, 128], BF16, tag="oh", name="oh")
                        nc.vector.tensor_scalar(out=oh[:], in0=iota_c[:],
                                                scalar1=dl_t[:, b:b + 1],
                                                scalar2=None, op0=AOP.is_equal)
                        nc.tensor.matmul(out=acc[:, w * INT:(w + 1) * INT],
                                         lhsT=oh[:],
                                         rhs=y_t[:, b * INT:(b + 1) * INT],
                                         start=(k == 0), stop=(k == NBW - 1))
                nc.vector.tensor_copy(out=xkj_all[:, ts(i, WPS * INT)],
                                      in_=acc[:])

            def body_e(i):
                def each(fn):
                    for n in range(ew):
                        fn(n)

                x0_t = {}
                for n in range(ew):
                    x0_t[n] = ep.tile([128, EDG], BF16, tag=f"x0_{n}",
                                      name=f"x0_{n}")
                    nc.sync.dma_start(out=x0_t[n][:],
                                      in_=x0T_d[:, ts(i * ew + n, EDG)])

                pjis = {}
                def _ji_mm(n):
                    pjis[n] = pb.tile([128, 512], F32, tag="pbig", name="pji")
                    nc.tensor.matmul(out=pjis[n][:, :EDG], lhsT=wji_c[:],
                                     rhs=x0_t[n][:], start=True, stop=True)
                each(_ji_mm)
                x_ji = {}
                def _ji_act(n):
                    x_ji[n] = ep.tile([128, EDG], BF16, tag=f"xji_{n}",
                                      name=f"xji_{n}")
                    nc.scalar.activation(out=x_ji[n][:], in_=pjis[n][:, :EDG],
                                         func=act, bias=bji)
                each(_ji_act)
                ptrs = {}
                def _tr(n):
                    ptrs[n] = ptp.tile([128, 512], BF16, tag="ptrb", name="ptr")
                    for w in range(WPS):
                        nc.tensor.transpose(
                            out=ptrs[n][:INT, w * 128:(w + 1) * 128],
                            in_=xkj_all[:, ts((i * ew + n) * WPS + w, INT)],
                            identity=ident_c[:])
                each(_tr)
                xkjT = {}
                def _trc(n):
                    xkjT[n] = ep.tile([INT, EDG], BF16, tag=f"xkT_{n}",
                                      name=f"xkT_{n}")
                    nc.vector.tensor_copy(out=xkjT[n][:], in_=ptrs[n][:INT, :EDG])
                each(_trc)
                pus = {}
                def _up_mm(n):
                    pus[n] = pb.tile([128, 512], F32, tag="pbig", name="pup")
                    nc.tensor.matmul(out=pus[n][:, :EDG], lhsT=wup_c[:],
                                     rhs=xkjT[n][:], start=True, stop=True)
                each(_up_mm)
                x2 = {}
                def _up_act(n):
                    xe = ep.tile([128, EDG], BF16, tag=f"xke_{n}", name=f"xke_{n}")
                    nc.scalar.activation(out=xe[:], in_=pus[n][:, :EDG], func=act)
                    x2[n] = ep.tile([128, EDG], BF16, tag=f"x2_{n}",
                                    name=f"x2_{n}")
                    nc.vector.tensor_tensor(out=x2[n][:], in0=x_ji[n][:],
                                            in1=xe[:], op=AOP.add)
                each(_up_act)

                def dense(w_c, srcs, bias_ap, tag):
                    outs = {}
                    ps_ = {}
                    def _mm(n):
                        ps_[n] = pb.tile([128, 512], F32, tag="pbig", name="pd")
                        nc.tensor.matmul(out=ps_[n][:, :EDG], lhsT=w_c[:],
                                         rhs=srcs[n][:], start=True, stop=True)
                    each(_mm)
                    def _act(n):
                        outs[n] = ep.tile([128, EDG], BF16, tag=f"{tag}_{n}",
                                          name=f"{tag}_{n}")
                        nc.scalar.activation(out=outs[n][:], in_=ps_[n][:, :EDG],
                                             func=act, bias=bias_ap)
                    each(_act)
                    return outs

                h1 = dense(wb1_c, x2, bb1, "h")
                h2 = dense(wb2_c, h1, bb2, "i2")
                x2b = {}
                def _add1(n):
                    x2b[n] = ep.tile([128, EDG], BF16, tag=f"x2b_{n}",
                                     name=f"x2b_{n}")
                    nc.vector.tensor_tensor(out=x2b[n][:], in0=x2[n][:],
                                            in1=h2[n][:], op=AOP.add)
                each(_add1)
                x2f = dense(wfin_c, x2b, bfin, "j2")
                x = {}
                def _skip(n):
                    x[n] = ep.tile([128, EDG], BF16, tag=f"x_{n}", name=f"x_{n}")
                    nc.gpsimd.tensor_tensor(out=x[n][:], in0=x0_t[n][:],
                                            in1=x2f[n][:], op=AOP.add)
                each(_skip)
                g1 = dense(wa11_c, x, ba11, "h")
                g2 = dense(wa12_c, g1, ba12, "i2")
                xa = {}
                def _add2(n):
                    xa[n] = ep.tile([128, EDG], BF16, tag=f"xa_{n}",
                                    name=f"xa_{n}")
                    nc.gpsimd.tensor_tensor(out=xa[n][:], in0=x[n][:],
                                            in1=g2[n][:], op=AOP.add)
                each(_add2)
                g3 = dense(wa21_c, xa, ba21, "h")
                g4 = dense(wa22_c, g3, ba22, "i2")
                def _fin(n):
                    xf = ep.tile([128, EDG], BF16, tag=f"xf_{n}", name=f"xf_{n}")
                    nc.gpsimd.tensor_tensor(out=xf[:], in0=xa[n][:],
                                            in1=g4[n][:], op=AOP.add)
                    nc.sync.dma_start(out=outT_d[:, ts(i * ew + n, EDG)],
                                      in_=xf[:])
                each(_fin)

            def run_loop(fn, n_iter):
                if n_iter == 1:
                    fn(0)
                else:
                    tc.For_i_unrolled(0, n_iter, 1, fn,
                                      max_unroll=min(unroll, n_iter))

            def run_all():
                run_loop(body_t, SC)
                # full python unroll: transpose reads of xkj_all need
                # compile-time offsets (no register offsets in ldweights)
                for j in range(SCE):
                    body_e(j)

            if repeats > 1:
                with tc.For_i(0, repeats) as r:
                    run_all()
            else:
                run_all()


# ----------------------------------------------------------------------------
# entry point
# ----------------------------------------------------------------------------

def kernel(**inputs):
    import sys
    if '/opt/trn_rl_repo' not in sys.path:
        sys.path.insert(0, '/opt/trn_rl_repo')
    from concourse import bacc
    from concourse.bass_utils import run_bass_kernel_spmd

    np_inputs = {k: np.asarray(v) for k, v in inputs.items()}
    x0 = np.asarray(np_inputs["x0"], np.float32)
    rbf = np.asarray(np_inputs["rbf"], np.float32)
    sbf = np.asarray(np_inputs["sbf"], np.float32)
    ide = np.asarray(np_inputs["id_expand_kj"], np.int64)
    idr = np.asarray(np_inputs["id_reduce_ji"], np.int64)

    cores, NBW, perm = _prep_core_arrays(x0, rbf, sbf, ide, idr)
    weights = _prep_weights(np_inputs)

    key = ("v3", NBW)
    if key not in _CACHE:
        nc = bacc.Bacc("TRN2", target_bir_lowering=False, debug=False,
                       enable_asserts=True, num_devices=NCORES)
        build_program(nc, ECa=EC, NBW=NBW, WPS=4, repeats=1, unroll=32, ew=2)
        nc.compile()
        _CACHE[key] = nc
    nc = _CACHE[key]

    in_maps = []
    for c in range(NCORES):
        m = dict(cores[c])
        m.update(weights)
        in_maps.append(m)
    res = run_bass_kernel_spmd(nc, in_maps, core_ids=list(range(NCORES)))
    outp = np.concatenate(
        [res.results[c]["outT"].T.astype(np.float32) for c in range(NCORES)],
        axis=0)
    out = np.empty((E, EMB), np.float32)
    out[perm] = outp
    return out
